# revision 3
# baseline (speedup 1.0000x reference)
"""3-layer GAT on Trainium2, 8 NeuronCores.

Strategy (graph/data parallel, dst-sharded):
  - Destination nodes are dealt round-robin (by degree rank) across 8 cores;
    each core owns LOCAL=6272 node slots (6250 real + 22 pad).
  - Per layer: each core computes h = x @ W for its nodes (plus the packed
    attention scalars s = h.a_src, d = h.a_dst via host-precomputed W@a
    columns), writes a 512B-row feature table slice, AllGathers the full
    table, then gathers h[src] rows per edge with GPSIMD dma_gather.
  - Edge layout is slot-major with lane = destination: chunk = one "slot"
    for 128 destinations of a tile.  That makes d (the per-dst attention
    term) a per-partition bias, the segment softmax a per-partition row
    operation, and the weighted aggregation a per-chunk diag(q) matmul
    accumulated in PSUM.  The denominator rides along as a constant-1
    column in the table rows.
  - dma_gather indices are int16, so the 50176-row table is addressed by
    two overlapping windows (lo = rows [0, 32768), hi = rows [BHI, RTOT));
    edges with sources in the overlap are assigned to whichever substream
    balances the per-tile slot counts (host-side optimization).
  - Softmax max-subtraction is skipped (max |e| ~ 9 on this model's data,
    exp is safe in fp32); padding slots gather a dummy row whose s = -1e30
    so exp gives exactly 0.
"""

import os
import sys

for _p in ("/opt/trn_rl_repo", "/opt/pypackages"):
    if os.path.isdir(_p) and _p not in sys.path:
        sys.path.insert(0, _p)

import ml_dtypes
import numpy as np

import concourse.bacc as bacc
import concourse.mybir as mybir
import concourse.tile as tile
from concourse.bass_utils import run_bass_kernel_spmd
from concourse.masks import make_identity

F32 = mybir.dt.float32
BF16 = mybir.dt.bfloat16
I16 = mybir.dt.int16
AF = mybir.ActivationFunctionType
ALU = mybir.AluOpType
AXL = mybir.AxisListType

P = 128
S_NEG = -1e30


def default_cfg():
    return dict(
        N=50000,
        C=8,
        DIMS=(128, 128, 64, 40),
        LO_WIN=32768,
        CAP=80,  # max gathered chunks per batch (SBUF budget)
        NEG_SLOPE=0.2,
        LRELU_NATIVE=False,  # native HW Lrelu table is inexact; use relu decomposition
    )


def _derived(cfg):
    N, C = cfg["N"], cfg["C"]
    assert N % C == 0
    tiles = (N // C + P - 1) // P
    local = tiles * P
    rtot = C * local
    bhi = max(0, rtot - cfg["LO_WIN"])
    return tiles, local, rtot, bhi


def preprocess(edge_index, cfg):
    """Host-side graph scheduling.  Returns a dict of per-core arrays and
    the (core-uniform) tile schedule."""
    N, C = cfg["N"], cfg["C"]
    TILES, LOCAL, RTOT, BHI = _derived(cfg)
    LO_WIN = cfg["LO_WIN"]

    src0 = np.asarray(edge_index[0], dtype=np.int64)
    dst0 = np.asarray(edge_index[1], dtype=np.int64)
    loop = np.arange(N, dtype=np.int64)
    src = np.concatenate([src0, loop])
    dst = np.concatenate([dst0, loop])
    E = src.shape[0]
    deg = np.bincount(dst, minlength=N)

    def deal(order):
        core_of = np.empty(N, np.int64)
        local_of = np.empty(N, np.int64)
        r = np.arange(N)
        core_of[order] = r % C
        local_of[order] = r // C
        return core_of, local_of

    # pass 1: rows from degree sort; pass 2 re-sorts with the fixed-lo count
    order = np.argsort(deg, kind="stable")
    core_of, local_of = deal(order)
    rows = core_of * LOCAL + local_of
    a = np.bincount(dst[rows[src] < BHI], minlength=N)
    order = np.lexsort((a, deg))
    core_of, local_of = deal(order)
    rows = core_of * LOCAL + local_of

    srow = rows[src]
    cat = np.where(srow < BHI, 0, np.where(srow < LO_WIN, 1, 2)).astype(np.int8)
    a = np.bincount(dst[cat == 0], minlength=N)
    f = np.bincount(dst[cat == 1], minlength=N)
    b = np.bincount(dst[cat == 2], minlength=N)
    assert np.all(a + f + b == deg)

    tile_of = local_of // P

    # per-tile substream depths (uniform across cores) via flex T-scan
    Ulo = np.zeros(TILES, np.int64)
    Uhi = np.zeros(TILES, np.int64)
    Tt = np.zeros(TILES, np.int64)
    for t in range(TILES):
        m = tile_of == t
        at, ft, bt = a[m], f[m], b[m]
        best = None
        lo_T = int(at.max()) if at.size else 0
        hi_T = int((at + ft).max()) if at.size else 0
        for T in range(lo_T, hi_T + 1):
            x = np.clip(T - at, 0, ft)
            lo = int((at + x).max())
            hi = int((bt + ft - x).max())
            if best is None or lo + hi < best[0]:
                best = (lo + hi, lo, hi, T)
        _, Ulo[t], Uhi[t], Tt[t] = best

    xflex = np.clip(Tt[tile_of] - a, 0, f)
    n_lo = a + xflex

    # per-edge slot assignment: order edges by (dst, category)
    eorder = np.lexsort((cat, dst))
    sd = dst[eorder]
    sval = srow[eorder]
    starts = np.zeros(N + 1, np.int64)
    np.cumsum(deg, out=starts[1:])
    posw = np.arange(E, dtype=np.int64) - starts[sd]
    is_lo_e = posw < n_lo[sd]
    slot = np.where(is_lo_e, posw, posw - n_lo[sd])

    cumlo = np.zeros(TILES + 1, np.int64)
    np.cumsum(Ulo, out=cumlo[1:])
    cumhi = np.zeros(TILES + 1, np.int64)
    np.cumsum(Uhi, out=cumhi[1:])
    LO_CH, HI_CH = int(cumlo[-1]), int(cumhi[-1])

    # dummy row: a pad row inside [BHI, min(LO_WIN, RTOT))
    n_real = N // C
    dummy = None
    if n_real < LOCAL:
        for c in range(C):
            r0 = c * LOCAL + n_real
            if BHI <= r0 < min(LO_WIN, RTOT):
                dummy = r0
                break
    assert dummy is not None, "no pad row available for the dummy entry"

    lane = local_of[sd] % P
    tl = tile_of[sd]
    cr = core_of[sd]

    lo_stream = np.full((C, LO_CH * P), dummy, np.int64)
    hi_stream = np.full((C, HI_CH * P), dummy - BHI, np.int64)
    ml = is_lo_e
    mh = ~is_lo_e
    lo_pos = (cumlo[tl[ml]] + slot[ml]) * P + lane[ml]
    hi_pos = (cumhi[tl[mh]] + slot[mh]) * P + lane[mh]
    lo_stream[cr[ml], lo_pos] = sval[ml]
    hi_stream[cr[mh], hi_pos] = sval[mh] - BHI
    # Pad lanes (locals >= n_real in the last tile) would otherwise gather
    # only dummy rows -> zero softmax denominator -> inf/NaN.  Point their
    # first slot at a real row so the denominator stays finite (their
    # output rows are discarded by the host anyway).
    n_real = N // C
    last = TILES - 1
    pl0 = n_real - last * P
    if pl0 < P:
        padlanes = np.arange(pl0, P)
        if Ulo[last] > 0:
            lo_stream[:, cumlo[last] * P + padlanes] = 0
        elif Uhi[last] > 0:
            hi_stream[:, cumhi[last] * P + padlanes] = RTOT - 1 - BHI
    assert lo_stream.min() >= 0 and lo_stream.max() < min(LO_WIN, RTOT)
    if HI_CH:
        assert hi_stream.min() >= 0 and hi_stream.max() < 32768

    def wrap(sarr):
        # stream position i -> [i % 16, i // 16]; the 16-partition block is
        # replicated to all 8 GPSIMD core groups (128 partitions).
        L = sarr.shape[1]
        if L == 0:
            return np.zeros((C, 128, 0), np.int16)
        w = np.ascontiguousarray(
            sarr.reshape(C, L // 16, 16).transpose(0, 2, 1)
        ).astype(np.int16)
        return np.tile(w, (1, 8, 1))

    # batches of tiles with bounded chunk totals
    batches = []
    t0 = 0
    while t0 < TILES:
        t1 = t0
        tot = 0
        while t1 < TILES and tot + Ulo[t1] + Uhi[t1] <= cfg["CAP"]:
            tot += Ulo[t1] + Uhi[t1]
            t1 += 1
        assert t1 > t0, f"tile {t0} exceeds CAP alone ({Ulo[t0]}+{Uhi[t0]})"
        batches.append((t0, t1))
        t0 = t1

    return dict(
        core_of=core_of,
        local_of=local_of,
        Ulo=Ulo,
        Uhi=Uhi,
        cumlo=cumlo,
        cumhi=cumhi,
        LO_CH=LO_CH,
        HI_CH=HI_CH,
        batches=batches,
        lo_idx=wrap(lo_stream),
        hi_idx=wrap(hi_stream),
        dummy=dummy,
        E_pad=(LO_CH + HI_CH) * P,
    )


def build_program(cfg, sched):
    """Emit the (core-uniform) Bass program."""
    N, C = cfg["N"], cfg["C"]
    DIMS = cfg["DIMS"]
    TILES, LOCAL, RTOT, BHI = _derived(cfg)
    Ulo, Uhi = sched["Ulo"], sched["Uhi"]
    cumlo, cumhi = sched["cumlo"], sched["cumhi"]
    LO_CH, HI_CH = sched["LO_CH"], sched["HI_CH"]
    batches = sched["batches"]
    CAP = cfg["CAP"]
    MAXU = int(max(Ulo[t] + Uhi[t] for t in range(TILES)))
    F_LAST = DIMS[3]

    nc = bacc.Bacc(
        "TRN2", target_bir_lowering=False, debug=False, num_devices=C,
        num_swdge_queues=4,
    )

    # ---- I/O ----
    x_t_in = nc.dram_tensor("x_t", [P, LOCAL], F32, kind="ExternalInput")
    w_in = [
        nc.dram_tensor(f"wfull{l}", [DIMS[l], DIMS[l + 1] + 2], F32,
                       kind="ExternalInput")
        for l in range(3)
    ]
    bb_in = [
        nc.dram_tensor(f"bb{l}", [P, DIMS[l + 1]], F32, kind="ExternalInput")
        for l in range(3)
    ]
    lo_in = nc.dram_tensor("lo_idx", [128, max(LO_CH * 8, 8)], I16,
                           kind="ExternalInput")
    hi_in = nc.dram_tensor("hi_idx", [128, max(HI_CH * 8, 8)], I16,
                           kind="ExternalInput")
    dums_in = nc.dram_tensor("dums", [3, 256], BF16, kind="ExternalInput")
    out_d = nc.dram_tensor("out_local", [LOCAL, F_LAST], F32,
                           kind="ExternalOutput")

    with tile.TileContext(nc) as tc:
        with tc.tile_pool(name="consts", bufs=1) as cp, \
             tc.tile_pool(name="dram", bufs=1, space="DRAM") as dp, \
             tc.tile_pool(name="work", bufs=3) as wp, \
             tc.tile_pool(name="mbuf", bufs=2) as mp, \
             tc.tile_pool(name="sq", bufs=6) as sqp, \
             tc.tile_pool(name="small", bufs=4) as rp, \
             tc.tile_pool(name="psA", bufs=2, space="PSUM") as psA, \
             tc.tile_pool(name="psB", bufs=2, space="PSUM") as psB, \
             tc.tile_pool(name="psC", bufs=3, space="PSUM") as psC:

            # ---- constants ----
            ident32 = cp.tile([P, P], F32, tag="ident32")
            make_identity(nc, ident32)
            identbf = cp.tile([P, P], BF16, tag="identbf")
            nc.vector.tensor_copy(identbf[:, :], ident32[:, :])

            w_sb = []
            bb_sb = []
            for l in range(3):
                wt = cp.tile([DIMS[l], DIMS[l + 1] + 2], F32, tag=f"w{l}",
                             name=f"w_sb{l}")
                nc.sync.dma_start(wt[:, :], w_in[l][:, :])
                w_sb.append(wt)
                bt = cp.tile([P, DIMS[l + 1]], F32, tag=f"bb{l}",
                             name=f"bb_sb{l}")
                nc.sync.dma_start(bt[:, :], bb_in[l][:, :])
                bb_sb.append(bt)

            lo_sb = cp.tile([128, max(LO_CH * 8, 8)], I16, tag="lo_sb")
            nc.sync.dma_start(lo_sb[:, :], lo_in[:, :])
            hi_sb = cp.tile([128, max(HI_CH * 8, 8)], I16, tag="hi_sb")
            nc.sync.dma_start(hi_sb[:, :], hi_in[:, :])
            dums_sb = cp.tile([3, 256], BF16, tag="dums_sb")
            nc.sync.dma_start(dums_sb[:, :], dums_in[:, :])

            d_all = [
                cp.tile([P, TILES], F32, tag=f"dall{l}", name=f"d_all{l}")
                for l in range(3)
            ]
            nxt = [
                cp.tile([P, TILES * DIMS[l + 1]], F32, tag=f"nxt{l}",
                        name=f"nxt{l}")
                for l in range(2)
            ]

            slices = [
                dp.tile([LOCAL, 256], BF16, tag=f"slice{l}", name=f"slice{l}")
                for l in range(3)
            ]
            # NOTE: addr_space="Shared" crashes NRT under the axon/PJRT
            # runtime (NRT_EXEC_UNIT_UNRECOVERABLE); Local-space output
            # works (bass warns it is slower).
            tables = [
                dp.tile([RTOT, 256], BF16, tag=f"table{l}", name=f"table{l}")
                for l in range(3)
            ]

            rg = [list(range(C))]

            STAGE = int(os.environ.get("GAT_STAGE", "99"))
            NLAYERS = min(3, max(1, STAGE // 10 + 1)) if STAGE < 99 else 3
            SUB = STAGE % 10 if STAGE < 99 else 9

            for l in range(NLAYERS):
                fi, fo = DIMS[l], DIMS[l + 1]

                # ---------- node phase ----------
                for t in range(TILES):
                    if l == 0:
                        xT = wp.tile([P, P], F32, tag="xT")
                        nc.sync.dma_start(
                            xT[:, :], x_t_in[:, t * P:(t + 1) * P])
                        xT_ap = xT[:fi, :]
                    else:
                        xv = nxt[l - 1][:, t * fi:(t + 1) * fi]
                        xT_ps = psA.tile([fi, P], F32, tag="xT_ps")
                        nc.tensor.transpose(xT_ps[:, :], xv, ident32[:, :])
                        xT = wp.tile([fi, P], F32, tag="xT")
                        nc.scalar.copy(xT[:, :], xT_ps[:, :])
                        xT_ap = xT[:, :]

                    h_ps = psB.tile([P, fo + 2], F32, tag="h_ps")
                    nc.tensor.matmul(h_ps[:, :], lhsT=xT_ap, rhs=w_sb[l][:, :],
                                     start=True, stop=True)

                    nc.vector.tensor_copy(
                        d_all[l][:, t:t + 1], h_ps[:, fo + 1:fo + 2])

                    stg = wp.tile([P, 256], BF16, tag="stg")
                    nc.scalar.copy(stg[:, 0:fo], h_ps[:, 0:fo])
                    nc.vector.memset(stg[:, fo:fo + 1], 1.0)
                    nc.vector.memset(stg[:, fo + 1:fo + 2], 0.0)
                    nc.vector.tensor_copy(
                        stg[:, fo + 2:fo + 4].bitcast(F32),
                        h_ps[:, fo:fo + 1])
                    nc.vector.memset(stg[:, fo + 4:256], 0.0)
                    nc.sync.dma_start(
                        slices[l][t * P:(t + 1) * P, :], stg[:, :])

                if l == NLAYERS - 1 and SUB < 1:
                    continue
                # ---------- dummy-row patch + all-gather ----------
                # Every core overwrites its pad row `n_real` with
                # [h=0.., s=-1e30, one=0]; only core DUMMY_CORE's copy is ever
                # gathered (as the padding target), the rest are inert.
                n_real = N // C
                nc.sync.dma_start(
                    slices[l][n_real:n_real + 1, :], dums_sb[l:l + 1, :])
                nc.gpsimd.collective_compute(
                    "AllGather",
                    ALU.bypass,
                    replica_groups=rg,
                    ins=[slices[l][:, :].opt()],
                    outs=[tables[l][:, :].opt()],
                )

                # ---------- edge phase ----------
                if l == NLAYERS - 1 and SUB < 2:
                    continue
                for bi, (t0, t1) in enumerate(batches):
                    nlo = int(cumlo[t1] - cumlo[t0])
                    nhi = int(cumhi[t1] - cumhi[t0])
                    nch = nlo + nhi
                    mb_t = mp.tile([P, CAP, 256], BF16, tag="mb")
                    if nlo:
                        nc.gpsimd.dma_gather(
                            out_ap=mb_t[:, 0:nlo, :],
                            in_ap=tables[l][:, :],
                            idxs_ap=lo_sb[:, int(cumlo[t0]) * 8:
                                          int(cumlo[t1]) * 8],
                            num_idxs=P * nlo,
                            num_idxs_reg=P * nlo,
                            elem_size=256,
                            single_packet=False,
                            queue_num=(2 * bi) % 4,
                        )
                    if nhi:
                        nc.gpsimd.dma_gather(
                            out_ap=mb_t[:, nlo:nch, :],
                            in_ap=tables[l][BHI:RTOT, :],
                            idxs_ap=hi_sb[:, int(cumhi[t0]) * 8:
                                          int(cumhi[t1]) * 8],
                            num_idxs=P * nhi,
                            num_idxs_reg=P * nhi,
                            elem_size=256,
                            single_packet=False,
                            queue_num=(2 * bi + 1) % 4,
                        )

                    if l == NLAYERS - 1 and SUB < 3:
                        continue
                    for t in range(t0, t1):
                        ulo = int(Ulo[t])
                        uhi = int(Uhi[t])
                        U = ulo + uhi
                        if U == 0:
                            continue
                        lob = int(cumlo[t] - cumlo[t0])
                        hib = nlo + int(cumhi[t] - cumhi[t0])
                        dcol = d_all[l][:, t:t + 1]

                        lre = rp.tile([P, MAXU], F32, tag="lre")
                        q_t = rp.tile([P, MAXU], F32, tag="q")
                        if cfg["LRELU_NATIVE"]:
                            if ulo:
                                nc.scalar.activation(
                                    lre[:, 0:ulo],
                                    mb_t[:, lob:lob + ulo,
                                         fo + 2:fo + 4].bitcast(F32),
                                    AF.Lrelu, bias=dcol, scale=1.0,
                                    alpha=cfg["NEG_SLOPE"])
                            if uhi:
                                nc.scalar.activation(
                                    lre[:, ulo:U],
                                    mb_t[:, hib:hib + uhi,
                                         fo + 2:fo + 4].bitcast(F32),
                                    AF.Lrelu, bias=dcol, scale=1.0,
                                    alpha=cfg["NEG_SLOPE"])
                            nc.scalar.activation(
                                q_t[:, 0:U], lre[:, 0:U], AF.Exp)
                        else:
                            # lrelu(z) = a*(z + r*relu(z)), a=NEG_SLOPE,
                            # r = (1-a)/a; fold `a` into Exp's scale.
                            zz = rp.tile([P, MAXU], F32, tag="zz")
                            if ulo:
                                sview = mb_t[:, lob:lob + ulo,
                                             fo + 2:fo + 4].bitcast(F32)
                                nc.scalar.activation(
                                    zz[:, 0:ulo], sview, AF.Identity,
                                    bias=dcol)
                                nc.scalar.activation(
                                    lre[:, 0:ulo], sview, AF.Relu, bias=dcol)
                            if uhi:
                                sview = mb_t[:, hib:hib + uhi,
                                             fo + 2:fo + 4].bitcast(F32)
                                nc.scalar.activation(
                                    zz[:, ulo:U], sview, AF.Identity,
                                    bias=dcol)
                                nc.scalar.activation(
                                    lre[:, ulo:U], sview, AF.Relu, bias=dcol)
                            a = cfg["NEG_SLOPE"]
                            nc.vector.scalar_tensor_tensor(
                                out=lre[:, 0:U], in0=lre[:, 0:U],
                                scalar=(1.0 - a) / a, in1=zz[:, 0:U],
                                op0=ALU.mult, op1=ALU.add)
                            nc.scalar.activation(
                                q_t[:, 0:U], lre[:, 0:U], AF.Exp, scale=a)

                        if l == NLAYERS - 1 and SUB < 4:
                            continue
                        acc = psC.tile([P, fo + 1], F32, tag="acc")
                        for u in range(U):
                            ch = (lob + u) if u < ulo else (hib + (u - ulo))
                            sq = sqp.tile([P, P], BF16, tag="sq")
                            nc.vector.tensor_scalar(
                                out=sq[:, :], in0=identbf[:, :],
                                scalar1=q_t[:, u:u + 1], scalar2=None,
                                op0=ALU.mult)
                            nc.tensor.matmul(
                                acc[:, :], lhsT=sq[:, :],
                                rhs=mb_t[:, ch, 0:fo + 1],
                                start=(u == 0), stop=(u == U - 1))

                        rc = rp.tile([P, 1], F32, tag="rc")
                        nc.vector.reciprocal(rc[:, :], acc[:, fo:fo + 1])
                        o_sb = wp.tile([P, fo], F32, tag="o_sb")
                        nc.vector.scalar_tensor_tensor(
                            out=o_sb[:, :], in0=acc[:, 0:fo], scalar=rc[:, :],
                            in1=bb_sb[l][:, :], op0=ALU.mult, op1=ALU.add)

                        if l < 2:
                            # SiLU via the exp table: x / (1 + exp(-x))
                            ex = wp.tile([P, fo], F32, tag="silu_e")
                            nc.scalar.activation(
                                ex[:, :], o_sb[:, :], AF.Exp, scale=-1.0)
                            nc.vector.tensor_scalar(
                                out=ex[:, :], in0=ex[:, :], scalar1=1.0,
                                scalar2=None, op0=ALU.add)
                            nc.vector.reciprocal(ex[:, :], ex[:, :])
                            nc.vector.tensor_tensor(
                                out=nxt[l][:, t * fo:(t + 1) * fo],
                                in0=o_sb[:, :], in1=ex[:, :], op=ALU.mult)
                        else:
                            mneg = rp.tile([P, 1], F32, tag="mneg")
                            nc.vector.tensor_reduce(
                                mneg[:, :], o_sb[:, :], axis=AXL.X,
                                op=ALU.max, negate=True)
                            ex2 = wp.tile([P, fo], F32, tag="ls_e")
                            se = rp.tile([P, 1], F32, tag="se")
                            nc.scalar.activation(
                                ex2[:, :], o_sb[:, :], AF.Exp, bias=mneg[:, :],
                                accum_out=se[:, :])
                            lse = rp.tile([P, 1], F32, tag="lse")
                            nc.scalar.activation(lse[:, :], se[:, :], AF.Ln)
                            fin = wp.tile([P, fo], F32, tag="fin")
                            nc.vector.tensor_scalar(
                                out=fin[:, :], in0=o_sb[:, :],
                                scalar1=mneg[:, :], scalar2=lse[:, :],
                                op0=ALU.add, op1=ALU.subtract)
                            nc.sync.dma_start(
                                out_d[t * P:(t + 1) * P, :], fin[:, :])

    nc.compile()
    return nc


def make_inputs(x, weights, cfg, sched):
    """Build the per-core in_maps."""
    N, C = cfg["N"], cfg["C"]
    TILES, LOCAL, RTOT, BHI = _derived(cfg)
    DIMS = cfg["DIMS"]
    core_of, local_of = sched["core_of"], sched["local_of"]

    x = np.asarray(x, np.float32)
    common = {}
    for l in range(3):
        W = np.asarray(weights[f"W{l}"], np.float64)
        a_s = np.asarray(weights[f"a_src{l}"], np.float64)
        a_d = np.asarray(weights[f"a_dst{l}"], np.float64)
        wfull = np.concatenate(
            [W, (W @ a_s)[:, None], (W @ a_d)[:, None]], axis=1)
        common[f"wfull{l}"] = np.ascontiguousarray(wfull, dtype=np.float32)
        b = np.asarray(weights[f"b{l}"], np.float32)
        common[f"bb{l}"] = np.ascontiguousarray(
            np.broadcast_to(b, (P, DIMS[l + 1])), dtype=np.float32)
    dums = np.zeros((3, 256), np.uint16)
    sneg = np.array([S_NEG], np.float32).view(np.uint16)
    for l in range(3):
        fo = DIMS[l + 1]
        dums[l, fo + 2:fo + 4] = sneg
    common["dums"] = dums.view(ml_dtypes.bfloat16).copy()

    in_maps = []
    for c in range(C):
        m = dict(common)
        nodes = np.where(core_of == c)[0]
        xt = np.zeros((P, LOCAL), np.float32)
        xt[:, local_of[nodes]] = x[nodes].T
        m["x_t"] = xt
        m["lo_idx"] = np.ascontiguousarray(
            sched["lo_idx"][c] if sched["LO_CH"] else
            np.zeros((128, 8), np.int16))
        m["hi_idx"] = np.ascontiguousarray(
            sched["hi_idx"][c] if sched["HI_CH"] else
            np.zeros((128, 8), np.int16))
        in_maps.append(m)
    return in_maps


LAST_EXEC_NS = None
LAST_RESULTS = None


def run(inputs, cfg=None, trace=False):
    global LAST_EXEC_NS, LAST_RESULTS
    cfg = cfg or default_cfg()
    N, C = cfg["N"], cfg["C"]
    TILES, LOCAL, RTOT, BHI = _derived(cfg)

    sched = preprocess(np.asarray(inputs["edge_index"]), cfg)
    nc = build_program(cfg, sched)
    in_maps = make_inputs(inputs["x"], inputs, cfg, sched)

    res = run_bass_kernel_spmd(
        nc, in_maps, core_ids=list(range(C)), trace=trace,
        stitch_traces=trace,
    )
    LAST_EXEC_NS = res.exec_time_ns
    LAST_RESULTS = res

    F_LAST = cfg["DIMS"][3]
    out = np.empty((N, F_LAST), np.float32)
    core_of, local_of = sched["core_of"], sched["local_of"]
    for c in range(C):
        nodes = np.where(core_of == c)[0]
        out[nodes] = res.results[c]["out_local"][local_of[nodes]]
    return out


def kernel(**inputs):
    return run(inputs, trace=bool(int(os.environ.get("GAT_TRACE", "0"))))



# revision 10
# speedup vs baseline: 1.0690x; 1.0690x over previous
"""3-layer GAT on Trainium2, 8 NeuronCores.

Strategy (graph/data parallel, dst-sharded):
  - Destination nodes are dealt round-robin (by degree rank) across 8 cores;
    each core owns LOCAL=6272 node slots (6250 real + 22 pad).
  - Per layer: each core computes h = x @ W for its nodes (plus the packed
    attention scalars s = h.a_src, d = h.a_dst via host-precomputed W@a
    columns), writes a 512B-row feature table slice, AllGathers the full
    table, then gathers h[src] rows per edge with GPSIMD dma_gather.
  - Edge layout is slot-major with lane = destination: chunk = one "slot"
    for 128 destinations of a tile.  That makes d (the per-dst attention
    term) a per-partition bias, the segment softmax a per-partition row
    operation, and the weighted aggregation a per-chunk diag(q) matmul
    accumulated in PSUM.  The denominator rides along as a constant-1
    column in the table rows.
  - dma_gather indices are int16, so the 50176-row table is addressed by
    two overlapping windows (lo = rows [0, 32768), hi = rows [BHI, RTOT));
    edges with sources in the overlap are assigned to whichever substream
    balances the per-tile slot counts (host-side optimization).
  - Softmax max-subtraction is skipped (max |e| ~ 9 on this model's data,
    exp is safe in fp32); padding slots gather a dummy row whose s = -1e30
    so exp gives exactly 0.
"""

import os
import sys

for _p in ("/opt/trn_rl_repo", "/opt/pypackages"):
    if os.path.isdir(_p) and _p not in sys.path:
        sys.path.insert(0, _p)

import ml_dtypes
import numpy as np

import concourse.bacc as bacc
import concourse.bass as bass
import concourse.mybir as mybir
import concourse.tile as tile
from concourse.bass_utils import run_bass_kernel_spmd
from concourse.masks import make_identity

F32 = mybir.dt.float32
BF16 = mybir.dt.bfloat16
I16 = mybir.dt.int16
AF = mybir.ActivationFunctionType
ALU = mybir.AluOpType
AXL = mybir.AxisListType

P = 128
S_NEG = -1e30


def default_cfg():
    return dict(
        N=50000,
        C=8,
        DIMS=(128, 128, 64, 40),
        LO_WIN=32768,
        CAP=64,  # max gathered chunks per batch (SBUF budget)
        NEG_SLOPE=0.2,
        ROWB=(256, 128, 128),  # bf16 slots per table row, per layer
    )


def _derived(cfg):
    N, C = cfg["N"], cfg["C"]
    assert N % C == 0
    tiles = (N // C + P - 1) // P
    local = tiles * P
    rtot = C * local
    bhi = max(0, rtot - cfg["LO_WIN"])
    return tiles, local, rtot, bhi


def preprocess(edge_index, cfg):
    """Host-side graph scheduling.  Returns a dict of per-core arrays and
    the (core-uniform) tile schedule."""
    N, C = cfg["N"], cfg["C"]
    TILES, LOCAL, RTOT, BHI = _derived(cfg)
    LO_WIN = cfg["LO_WIN"]

    src0 = np.asarray(edge_index[0], dtype=np.int64)
    dst0 = np.asarray(edge_index[1], dtype=np.int64)
    loop = np.arange(N, dtype=np.int64)
    src = np.concatenate([src0, loop])
    dst = np.concatenate([dst0, loop])
    E = src.shape[0]
    deg = np.bincount(dst, minlength=N)

    def deal(order):
        core_of = np.empty(N, np.int64)
        local_of = np.empty(N, np.int64)
        r = np.arange(N)
        core_of[order] = r % C
        local_of[order] = r // C
        return core_of, local_of

    # pass 1: rows from degree sort; pass 2 re-sorts with the fixed-lo count
    order = np.argsort(deg, kind="stable")
    core_of, local_of = deal(order)
    rows = core_of * LOCAL + local_of
    a = np.bincount(dst[rows[src] < BHI], minlength=N)
    order = np.lexsort((a, deg))
    core_of, local_of = deal(order)
    rows = core_of * LOCAL + local_of

    srow = rows[src]
    cat = np.where(srow < BHI, 0, np.where(srow < LO_WIN, 1, 2)).astype(np.int8)
    a = np.bincount(dst[cat == 0], minlength=N)
    f = np.bincount(dst[cat == 1], minlength=N)
    b = np.bincount(dst[cat == 2], minlength=N)
    assert np.all(a + f + b == deg)

    tile_of = local_of // P

    # per-tile substream depths (uniform across cores) via flex T-scan
    Ulo = np.zeros(TILES, np.int64)
    Uhi = np.zeros(TILES, np.int64)
    Tt = np.zeros(TILES, np.int64)
    for t in range(TILES):
        m = tile_of == t
        at, ft, bt = a[m], f[m], b[m]
        best = None
        lo_T = int(at.max()) if at.size else 0
        hi_T = int((at + ft).max()) if at.size else 0
        for T in range(lo_T, hi_T + 1):
            x = np.clip(T - at, 0, ft)
            lo = int((at + x).max())
            hi = int((bt + ft - x).max())
            if best is None or lo + hi < best[0]:
                best = (lo + hi, lo, hi, T)
        _, Ulo[t], Uhi[t], Tt[t] = best

    xflex = np.clip(Tt[tile_of] - a, 0, f)
    n_lo = a + xflex

    # per-edge slot assignment: order edges by (dst, category)
    eorder = np.lexsort((cat, dst))
    sd = dst[eorder]
    sval = srow[eorder]
    starts = np.zeros(N + 1, np.int64)
    np.cumsum(deg, out=starts[1:])
    posw = np.arange(E, dtype=np.int64) - starts[sd]
    is_lo_e = posw < n_lo[sd]
    slot = np.where(is_lo_e, posw, posw - n_lo[sd])

    cumlo = np.zeros(TILES + 1, np.int64)
    np.cumsum(Ulo, out=cumlo[1:])
    cumhi = np.zeros(TILES + 1, np.int64)
    np.cumsum(Uhi, out=cumhi[1:])
    LO_CH, HI_CH = int(cumlo[-1]), int(cumhi[-1])

    # dummy row: a pad row inside [BHI, min(LO_WIN, RTOT))
    n_real = N // C
    dummy = None
    if n_real < LOCAL:
        for c in range(C):
            r0 = c * LOCAL + n_real
            if BHI <= r0 < min(LO_WIN, RTOT):
                dummy = r0
                break
    assert dummy is not None, "no pad row available for the dummy entry"

    lane = local_of[sd] % P
    tl = tile_of[sd]
    cr = core_of[sd]

    lo_stream = np.full((C, LO_CH * P), dummy, np.int64)
    hi_stream = np.full((C, HI_CH * P), dummy - BHI, np.int64)
    ml = is_lo_e
    mh = ~is_lo_e
    lo_pos = (cumlo[tl[ml]] + slot[ml]) * P + lane[ml]
    hi_pos = (cumhi[tl[mh]] + slot[mh]) * P + lane[mh]
    lo_stream[cr[ml], lo_pos] = sval[ml]
    hi_stream[cr[mh], hi_pos] = sval[mh] - BHI
    # Pad lanes (locals >= n_real in the last tile) would otherwise gather
    # only dummy rows -> zero softmax denominator -> inf/NaN.  Point their
    # first slot at a real row so the denominator stays finite (their
    # output rows are discarded by the host anyway).
    n_real = N // C
    last = TILES - 1
    pl0 = n_real - last * P
    if pl0 < P:
        padlanes = np.arange(pl0, P)
        if Ulo[last] > 0:
            lo_stream[:, cumlo[last] * P + padlanes] = 0
        elif Uhi[last] > 0:
            hi_stream[:, cumhi[last] * P + padlanes] = RTOT - 1 - BHI
    assert lo_stream.min() >= 0 and lo_stream.max() < min(LO_WIN, RTOT)
    if HI_CH:
        assert hi_stream.min() >= 0 and hi_stream.max() < 32768

    def wrap(sarr):
        # stream position i -> [i % 16, i // 16]; the 16-partition block is
        # replicated to all 8 GPSIMD core groups (128 partitions).
        L = sarr.shape[1]
        if L == 0:
            return np.zeros((C, 128, 0), np.int16)
        w = np.ascontiguousarray(
            sarr.reshape(C, L // 16, 16).transpose(0, 2, 1)
        ).astype(np.int16)
        return np.tile(w, (1, 8, 1))

    # batches of tiles with bounded chunk totals
    batches = []
    t0 = 0
    while t0 < TILES:
        t1 = t0
        tot = 0
        while t1 < TILES and tot + Ulo[t1] + Uhi[t1] <= cfg["CAP"]:
            tot += Ulo[t1] + Uhi[t1]
            t1 += 1
        assert t1 > t0, f"tile {t0} exceeds CAP alone ({Ulo[t0]}+{Uhi[t0]})"
        batches.append((t0, t1))
        t0 = t1

    return dict(
        core_of=core_of,
        local_of=local_of,
        Ulo=Ulo,
        Uhi=Uhi,
        cumlo=cumlo,
        cumhi=cumhi,
        LO_CH=LO_CH,
        HI_CH=HI_CH,
        batches=batches,
        lo_idx=wrap(lo_stream),
        hi_idx=wrap(hi_stream),
        dummy=dummy,
        E_pad=(LO_CH + HI_CH) * P,
    )


def build_program(cfg, sched):
    """Emit the (core-uniform) Bass program."""
    N, C = cfg["N"], cfg["C"]
    DIMS = cfg["DIMS"]
    TILES, LOCAL, RTOT, BHI = _derived(cfg)
    Ulo, Uhi = sched["Ulo"], sched["Uhi"]
    cumlo, cumhi = sched["cumlo"], sched["cumhi"]
    LO_CH, HI_CH = sched["LO_CH"], sched["HI_CH"]
    batches = sched["batches"]
    CAP = cfg["CAP"]
    MAXU = int(max(Ulo[t] + Uhi[t] for t in range(TILES)))
    F_LAST = DIMS[3]

    nc = bacc.Bacc(
        "TRN2", target_bir_lowering=False, debug=False, num_devices=C,
        num_swdge_queues=4,
    )

    # ---- I/O ----
    x_t_in = nc.dram_tensor("x_t", [P, LOCAL], F32, kind="ExternalInput")
    w_in = [
        nc.dram_tensor(f"wfull{l}", [DIMS[l], DIMS[l + 1] + 2], F32,
                       kind="ExternalInput")
        for l in range(3)
    ]
    bb_in = [
        nc.dram_tensor(f"bb{l}", [P, DIMS[l + 1]], F32, kind="ExternalInput")
        for l in range(3)
    ]
    lo_in = nc.dram_tensor("lo_idx", [128, max(LO_CH * 8, 8)], I16,
                           kind="ExternalInput")
    hi_in = nc.dram_tensor("hi_idx", [128, max(HI_CH * 8, 8)], I16,
                           kind="ExternalInput")
    dums_in = nc.dram_tensor("dums", [3, 256], BF16, kind="ExternalInput")
    out_d = nc.dram_tensor("out_local", [LOCAL, F_LAST], F32,
                           kind="ExternalOutput")

    ROWB = cfg["ROWB"]

    with tile.TileContext(nc) as tc:
        with tc.tile_pool(name="consts", bufs=1) as cp, \
             tc.tile_pool(name="dram", bufs=1, space="DRAM") as dp, \
             tc.tile_pool(name="work", bufs=3) as wp, \
             tc.tile_pool(name="small", bufs=4) as rp, \
             tc.tile_pool(name="psA", bufs=2, space="PSUM") as psA, \
             tc.tile_pool(name="psB", bufs=2, space="PSUM") as psB, \
             tc.tile_pool(name="psC", bufs=3, space="PSUM") as psC:

            # ---- constants ----
            ident32 = cp.tile([P, P], F32, tag="ident32")
            make_identity(nc, ident32)
            identbf = cp.tile([P, P], BF16, tag="identbf")
            nc.vector.tensor_copy(identbf[:, :], ident32[:, :])

            w_sb = []
            bb_sb = []
            for l in range(3):
                wt = cp.tile([DIMS[l], DIMS[l + 1] + 2], F32, tag=f"w{l}",
                             name=f"w_sb{l}")
                nc.sync.dma_start(wt[:, :], w_in[l][:, :])
                w_sb.append(wt)
                bt = cp.tile([P, DIMS[l + 1]], F32, tag=f"bb{l}",
                             name=f"bb_sb{l}")
                nc.sync.dma_start(bt[:, :], bb_in[l][:, :])
                bb_sb.append(bt)

            lo_sb = cp.tile([128, max(LO_CH * 8, 8)], I16, tag="lo_sb")
            nc.sync.dma_start(lo_sb[:, :], lo_in[:, :])
            hi_sb = cp.tile([128, max(HI_CH * 8, 8)], I16, tag="hi_sb")
            nc.sync.dma_start(hi_sb[:, :], hi_in[:, :])
            dums_sb = cp.tile([3, 256], BF16, tag="dums_sb")
            nc.sync.dma_start(dums_sb[:, :], dums_in[:, :])

            d_all = [
                cp.tile([P, TILES], F32, tag=f"dall{l}", name=f"d_all{l}")
                for l in range(3)
            ]
            nxt = [
                cp.tile([P, TILES * DIMS[l + 1]], F32, tag=f"nxt{l}",
                        name=f"nxt{l}")
                for l in range(2)
            ]

            slices = [
                dp.tile([LOCAL, ROWB[l]], BF16, tag=f"slice{l}",
                        name=f"slice{l}")
                for l in range(3)
            ]
            # NOTE: addr_space="Shared" crashes NRT under the axon/PJRT
            # runtime (NRT_EXEC_UNIT_UNRECOVERABLE); Local-space output
            # works (bass warns it is slower).
            tables = [
                dp.tile([RTOT, ROWB[l]], BF16, tag=f"table{l}",
                        name=f"table{l}")
                for l in range(3)
            ]

            rg = [list(range(C))]

            STAGE = int(os.environ.get("GAT_STAGE", "99"))
            NLAYERS = min(3, max(1, STAGE // 10 + 1)) if STAGE < 99 else 3
            SUB = STAGE % 10 if STAGE < 99 else 9

            for l in range(NLAYERS):
                fi, fo = DIMS[l], DIMS[l + 1]

                # ---------- node phase ----------
                for t in range(TILES):
                    if l == 0:
                        xT = wp.tile([P, P], F32, tag="xT")
                        nc.sync.dma_start(
                            xT[:, :], x_t_in[:, t * P:(t + 1) * P])
                        xT_ap = xT[:fi, :]
                    else:
                        xv = nxt[l - 1][:, t * fi:(t + 1) * fi]
                        xT_ps = psA.tile([fi, P], F32, tag="xT_ps")
                        nc.tensor.transpose(xT_ps[:, :], xv, ident32[:, :])
                        xT = wp.tile([fi, P], F32, tag="xT")
                        nc.scalar.copy(xT[:, :], xT_ps[:, :])
                        xT_ap = xT[:, :]

                    h_ps = psB.tile([P, fo + 2], F32, tag="h_ps")
                    nc.tensor.matmul(h_ps[:, :], lhsT=xT_ap, rhs=w_sb[l][:, :],
                                     start=True, stop=True)

                    nc.vector.tensor_copy(
                        d_all[l][:, t:t + 1], h_ps[:, fo + 1:fo + 2])

                    stg = wp.tile([P, ROWB[l]], BF16, tag=f"stg{l}")
                    nc.scalar.copy(stg[:, 0:fo], h_ps[:, 0:fo])
                    nc.vector.memset(stg[:, fo:fo + 1], 1.0)
                    nc.vector.memset(stg[:, fo + 1:fo + 2], 0.0)
                    nc.vector.tensor_copy(
                        stg[:, fo + 2:fo + 4].bitcast(F32),
                        h_ps[:, fo:fo + 1])
                    nc.vector.memset(stg[:, fo + 4:ROWB[l]], 0.0)
                    nc.sync.dma_start(
                        slices[l][t * P:(t + 1) * P, :], stg[:, :])

                if l == NLAYERS - 1 and SUB < 1:
                    continue
                # ---------- dummy-row patch + all-gather ----------
                # Every core overwrites its pad row `n_real` with
                # [h=0.., s=-1e30, one=0]; only core DUMMY_CORE's copy is ever
                # gathered (as the padding target), the rest are inert.
                n_real = N // C
                nc.sync.dma_start(
                    slices[l][n_real:n_real + 1, :],
                    dums_sb[l:l + 1, 0:ROWB[l]])
                nc.gpsimd.collective_compute(
                    "AllGather",
                    ALU.bypass,
                    replica_groups=rg,
                    ins=[slices[l][:, :].opt()],
                    outs=[tables[l][:, :].opt()],
                )

                # ---------- edge phase ----------
                if l == NLAYERS - 1 and SUB < 2:
                    continue
                with tc.tile_pool(name=f"mbuf{l}", bufs=2) as mp, \
                     tc.tile_pool(name=f"ebuf{l}", bufs=2) as ep:
                  for bi, (t0, t1) in enumerate(batches):
                    nlo = int(cumlo[t1] - cumlo[t0])
                    nhi = int(cumhi[t1] - cumhi[t0])
                    nch = nlo + nhi
                    mb_t = mp.tile([P, CAP, ROWB[l]], BF16, tag="mb")
                    if nlo:
                        nc.gpsimd.dma_gather(
                            out_ap=mb_t[:, 0:nlo, :],
                            in_ap=tables[l][:, :],
                            idxs_ap=lo_sb[:, int(cumlo[t0]) * 8:
                                          int(cumlo[t1]) * 8],
                            num_idxs=P * nlo,
                            num_idxs_reg=P * nlo,
                            elem_size=ROWB[l],
                            single_packet=False,
                            queue_num=(2 * bi) % 4,
                        )
                    if nhi:
                        nc.gpsimd.dma_gather(
                            out_ap=mb_t[:, nlo:nch, :],
                            in_ap=tables[l][BHI:RTOT, :],
                            idxs_ap=hi_sb[:, int(cumhi[t0]) * 8:
                                          int(cumhi[t1]) * 8],
                            num_idxs=P * nhi,
                            num_idxs_reg=P * nhi,
                            elem_size=ROWB[l],
                            single_packet=False,
                            queue_num=(2 * bi + 1) % 4,
                        )

                    if l == NLAYERS - 1 and SUB < 3:
                        continue

                    # per-(tile, substream) biased s extraction; zz/lre
                    # accumulate the whole batch in chunk order.
                    zzb = ep.tile([P, CAP], F32, tag="zzb")
                    lreb = ep.tile([P, CAP], F32, tag="lreb")
                    qb = ep.tile([P, CAP], BF16, tag="qb")
                    sqall = ep.tile([P, CAP * P], BF16, tag="sqall")
                    for t in range(t0, t1):
                        ulo = int(Ulo[t])
                        uhi = int(Uhi[t])
                        if ulo + uhi == 0:
                            continue
                        lob = int(cumlo[t] - cumlo[t0])
                        hib = nlo + int(cumhi[t] - cumhi[t0])
                        dcol = d_all[l][:, t:t + 1]
                        if ulo:
                            sview = mb_t[:, lob:lob + ulo,
                                         fo + 2:fo + 4].bitcast(F32)
                            nc.scalar.activation(
                                zzb[:, lob:lob + ulo], sview, AF.Identity,
                                bias=dcol)
                            nc.scalar.activation(
                                lreb[:, lob:lob + ulo], sview, AF.Relu,
                                bias=dcol)
                        if uhi:
                            sview = mb_t[:, hib:hib + uhi,
                                         fo + 2:fo + 4].bitcast(F32)
                            nc.scalar.activation(
                                zzb[:, hib:hib + uhi], sview, AF.Identity,
                                bias=dcol)
                            nc.scalar.activation(
                                lreb[:, hib:hib + uhi], sview, AF.Relu,
                                bias=dcol)

                    # lrelu(z) = a*(z + r*relu(z)), r=(1-a)/a; fold `a`
                    # into Exp's scale.  One op per batch.
                    a = cfg["NEG_SLOPE"]
                    nc.vector.scalar_tensor_tensor(
                        out=lreb[:, 0:nch], in0=lreb[:, 0:nch],
                        scalar=(1.0 - a) / a, in1=zzb[:, 0:nch],
                        op0=ALU.mult, op1=ALU.add)
                    nc.scalar.activation(
                        qb[:, 0:nch], lreb[:, 0:nch], AF.Exp, scale=a)

                    # batched diag build: sqall[p, c*P+j] =
                    #   identbf[p, j] * qb[p, c]   (stride-0 broadcasts)
                    vi = identbf[:, :]
                    vq = qb[:, :]
                    vo = sqall[:, :]
                    nc.vector.tensor_tensor(
                        out=bass.AP(vo.tensor, vo.offset,
                                    [list(vo.ap[0]), [P, nch], [1, P]]),
                        in0=bass.AP(vi.tensor, vi.offset,
                                    [list(vi.ap[0]), [0, nch], [1, P]]),
                        in1=bass.AP(vq.tensor, vq.offset,
                                    [list(vq.ap[0]), [1, nch], [0, P]]),
                        op=ALU.mult)

                    for t in range(t0, t1):
                        ulo = int(Ulo[t])
                        uhi = int(Uhi[t])
                        U = ulo + uhi
                        if U == 0:
                            continue
                        lob = int(cumlo[t] - cumlo[t0])
                        hib = nlo + int(cumhi[t] - cumhi[t0])

                        if l == NLAYERS - 1 and SUB < 4:
                            continue
                        acc = psC.tile([P, fo + 1], F32, tag="acc")
                        for u in range(U):
                            ch = (lob + u) if u < ulo else (hib + (u - ulo))
                            nc.tensor.matmul(
                                acc[:, :],
                                lhsT=sqall[:, ch * P:(ch + 1) * P],
                                rhs=mb_t[:, ch, 0:fo + 1],
                                start=(u == 0), stop=(u == U - 1))

                        rc = rp.tile([P, 1], F32, tag="rc")
                        nc.vector.reciprocal(rc[:, :], acc[:, fo:fo + 1])
                        o_sb = wp.tile([P, fo], F32, tag="o_sb")
                        nc.vector.scalar_tensor_tensor(
                            out=o_sb[:, :], in0=acc[:, 0:fo], scalar=rc[:, :],
                            in1=bb_sb[l][:, :], op0=ALU.mult, op1=ALU.add)

                        if l < 2:
                            # SiLU via the exp table: x / (1 + exp(-x))
                            ex = wp.tile([P, fo], F32, tag="silu_e")
                            nc.scalar.activation(
                                ex[:, :], o_sb[:, :], AF.Exp, scale=-1.0)
                            nc.vector.tensor_scalar(
                                out=ex[:, :], in0=ex[:, :], scalar1=1.0,
                                scalar2=None, op0=ALU.add)
                            nc.vector.reciprocal(ex[:, :], ex[:, :])
                            nc.vector.tensor_tensor(
                                out=nxt[l][:, t * fo:(t + 1) * fo],
                                in0=o_sb[:, :], in1=ex[:, :], op=ALU.mult)
                        else:
                            mneg = rp.tile([P, 1], F32, tag="mneg")
                            nc.vector.tensor_reduce(
                                mneg[:, :], o_sb[:, :], axis=AXL.X,
                                op=ALU.max, negate=True)
                            ex2 = wp.tile([P, fo], F32, tag="ls_e")
                            se = rp.tile([P, 1], F32, tag="se")
                            nc.scalar.activation(
                                ex2[:, :], o_sb[:, :], AF.Exp, bias=mneg[:, :],
                                accum_out=se[:, :])
                            lse = rp.tile([P, 1], F32, tag="lse")
                            nc.scalar.activation(lse[:, :], se[:, :], AF.Ln)
                            fin = wp.tile([P, fo], F32, tag="fin")
                            nc.vector.tensor_scalar(
                                out=fin[:, :], in0=o_sb[:, :],
                                scalar1=mneg[:, :], scalar2=lse[:, :],
                                op0=ALU.add, op1=ALU.subtract)
                            nc.sync.dma_start(
                                out_d[t * P:(t + 1) * P, :], fin[:, :])

    nc.compile()
    return nc


def make_inputs(x, weights, cfg, sched):
    """Build the per-core in_maps."""
    N, C = cfg["N"], cfg["C"]
    TILES, LOCAL, RTOT, BHI = _derived(cfg)
    DIMS = cfg["DIMS"]
    core_of, local_of = sched["core_of"], sched["local_of"]

    x = np.asarray(x, np.float32)
    common = {}
    for l in range(3):
        W = np.asarray(weights[f"W{l}"], np.float64)
        a_s = np.asarray(weights[f"a_src{l}"], np.float64)
        a_d = np.asarray(weights[f"a_dst{l}"], np.float64)
        wfull = np.concatenate(
            [W, (W @ a_s)[:, None], (W @ a_d)[:, None]], axis=1)
        common[f"wfull{l}"] = np.ascontiguousarray(wfull, dtype=np.float32)
        b = np.asarray(weights[f"b{l}"], np.float32)
        common[f"bb{l}"] = np.ascontiguousarray(
            np.broadcast_to(b, (P, DIMS[l + 1])), dtype=np.float32)
    dums = np.zeros((3, 256), np.uint16)
    sneg = np.array([S_NEG], np.float32).view(np.uint16)
    for l in range(3):
        fo = DIMS[l + 1]
        dums[l, fo + 2:fo + 4] = sneg
    common["dums"] = dums.view(ml_dtypes.bfloat16).copy()

    in_maps = []
    for c in range(C):
        m = dict(common)
        nodes = np.where(core_of == c)[0]
        xt = np.zeros((P, LOCAL), np.float32)
        xt[:, local_of[nodes]] = x[nodes].T
        m["x_t"] = xt
        m["lo_idx"] = np.ascontiguousarray(
            sched["lo_idx"][c] if sched["LO_CH"] else
            np.zeros((128, 8), np.int16))
        m["hi_idx"] = np.ascontiguousarray(
            sched["hi_idx"][c] if sched["HI_CH"] else
            np.zeros((128, 8), np.int16))
        in_maps.append(m)
    return in_maps


LAST_EXEC_NS = None
LAST_RESULTS = None


def run(inputs, cfg=None, trace=False):
    global LAST_EXEC_NS, LAST_RESULTS
    cfg = cfg or default_cfg()
    N, C = cfg["N"], cfg["C"]
    TILES, LOCAL, RTOT, BHI = _derived(cfg)

    sched = preprocess(np.asarray(inputs["edge_index"]), cfg)
    nc = build_program(cfg, sched)
    in_maps = make_inputs(inputs["x"], inputs, cfg, sched)

    res = run_bass_kernel_spmd(
        nc, in_maps, core_ids=list(range(C)), trace=trace,
        stitch_traces=trace,
    )
    LAST_EXEC_NS = res.exec_time_ns
    LAST_RESULTS = res

    F_LAST = cfg["DIMS"][3]
    out = np.empty((N, F_LAST), np.float32)
    core_of, local_of = sched["core_of"], sched["local_of"]
    for c in range(C):
        nodes = np.where(core_of == c)[0]
        out[nodes] = res.results[c]["out_local"][local_of[nodes]]
    return out


def kernel(**inputs):
    return run(inputs, trace=bool(int(os.environ.get("GAT_TRACE", "0"))))



# revision 25
# speedup vs baseline: 1.7321x; 1.6203x over previous
"""3-layer GAT on Trainium2, 8 NeuronCores.

Strategy (graph/data parallel, dst-sharded).  The kernel is bound by
dma_gather's per-row DMA descriptors (HBM random-read latency), so the
design minimizes gathered rows and keeps all 4 SWDGE queue rings loaded:

  - Destination nodes are dealt round-robin (by degree rank) across 8
    cores; each core owns LOCAL=6272 node slots (6250 real + 22 pad).
  - Per layer: each core computes h = x @ W for its nodes (plus attention
    scalars s = h.a_src, d = h.a_dst via host-precomputed W@a columns),
    stages a bf16 row [h | 1 | 0 | s_f32] per node (512B rows for
    fo=128, 256B otherwise), and publishes it via FOUR partial
    AllGathers (chunked by node-tile range) that overlap the previous
    layer's edge phase; next-layer node tiles are emitted inline in the
    edge loop so the engines actually interleave them.
  - Edge layout is slot-major with lane = destination: chunk = one slot
    for the 128 destinations of a tile, so d is a per-partition bias,
    the segment softmax is a per-partition row op, and aggregation is a
    per-chunk diag(q) matmul accumulated in PSUM (denominator = the
    constant-1 row column).  All per-chunk diag matrices of a batch are
    built in ONE DVE tensor_tensor using stride-0 broadcast APs.
  - Self-loop edges are never gathered: their rows are core-local, added
    per tile as one extra matmul (diag(q_self) x [h | 1] re-read densely
    from the slice).
  - dma_gather indices are int16, so the 50176-row table is addressed
    through THREE overlapping 32768-row windows (bases 0, BHI/2, BHI =
    plain in_ap offsets); each edge is assigned to a window stream by a
    per-tile balance optimizer, cutting slot padding to ~25% over the
    single-stream floor.  Gathers are split in half and striped over the
    4 SWDGE queues: SDMA engines round-robin rings at packet granularity,
    which overlaps the HBM read latency of different rings.
  - Softmax max-subtraction is skipped (max |e| ~ 9 here, exp is safe in
    fp32); padding slots gather a dummy pad row whose s = -1e30 (q = 0)
    and one = 1.0 (keeps the patched row's self-loop denominator at 1).
"""

import os
import sys

for _p in ("/opt/trn_rl_repo", "/opt/pypackages"):
    if os.path.isdir(_p) and _p not in sys.path:
        sys.path.insert(0, _p)

import ml_dtypes
import numpy as np

import concourse.bacc as bacc
import concourse.bass as bass
import concourse.mybir as mybir
import concourse.tile as tile
from concourse.bass_utils import run_bass_kernel_spmd
from concourse.masks import make_identity

F32 = mybir.dt.float32
BF16 = mybir.dt.bfloat16
I16 = mybir.dt.int16
AF = mybir.ActivationFunctionType
ALU = mybir.AluOpType
AXL = mybir.AxisListType

P = 128
S_NEG = -1e30


def default_cfg():
    return dict(
        N=50000,
        C=8,
        DIMS=(128, 128, 64, 40),
        LO_WIN=32768,
        CAP=64,  # max gathered chunks per batch (SBUF budget)
        NEG_SLOPE=0.2,
        ROWB=(256, 128, 128),  # bf16 slots per table row, per layer
    )


def _derived(cfg):
    N, C = cfg["N"], cfg["C"]
    assert N % C == 0
    tiles = (N // C + P - 1) // P
    local = tiles * P
    rtot = C * local
    bhi = max(0, rtot - cfg["LO_WIN"])
    return tiles, local, rtot, bhi


# AllGather chunking: the table is built by 4 partial AllGathers over
# node-tile ranges so each chunk can launch as soon as its node tiles are
# staged (hiding the collective under the previous layer's edge phase).
# Each chunk's output block is [core0-rows | core1-rows | ...]; the
# pad-bearing chunk (last tile range) is placed SECOND in table order so
# some core's pad row lands in the lo/hi index overlap [BHI, LO_WIN) and
# can serve as the dummy gather target.
AG_BOUNDS = (0, 13, 26, 38, 49)  # node-tile range boundaries
AG_TORD = (0, 3, 1, 2)  # range index -> position in table order


def _ag_blocks(cfg):
    """Returns [(lo_local, hi_local, table_base)] indexed by range id."""
    C = cfg["C"]
    ranges = [
        (AG_BOUNDS[i] * P, AG_BOUNDS[i + 1] * P) for i in range(4)
    ]
    base = 0
    tb = {}
    for ri in AG_TORD:
        lo, hi = ranges[ri]
        tb[ri] = base
        base += (hi - lo) * C
    return [(lo, hi, tb[ri]) for ri, (lo, hi) in enumerate(ranges)]


def _row_of(core, local, cfg):
    """Table row of (core, local) under the chunked-AllGather layout."""
    core, local = np.broadcast_arrays(np.asarray(core), np.asarray(local))
    row = np.empty(local.shape, np.int64)
    for lo, hi, tbase in _ag_blocks(cfg):
        m = (local >= lo) & (local < hi)
        row[m] = tbase + core[m] * (hi - lo) + (local[m] - lo)
    return row


def preprocess(edge_index, cfg):
    """Host-side graph scheduling.  Returns a dict of per-core arrays and
    the (core-uniform) tile schedule."""
    N, C = cfg["N"], cfg["C"]
    TILES, LOCAL, RTOT, BHI = _derived(cfg)
    LO_WIN = cfg["LO_WIN"]

    # Self-loop edges (the reference appends one per node) are NOT put in
    # the gather streams: their source rows are core-local, so the kernel
    # adds the q_self * [h | 1] contribution with one extra per-tile
    # matmul instead of gathering 50k rows.
    src = np.asarray(edge_index[0], dtype=np.int64)
    dst = np.asarray(edge_index[1], dtype=np.int64)
    E = src.shape[0]
    deg = np.bincount(dst, minlength=N)

    def deal(order):
        core_of = np.empty(N, np.int64)
        local_of = np.empty(N, np.int64)
        r = np.arange(N)
        core_of[order] = r % C
        local_of[order] = r // C
        return core_of, local_of

    # pass 1: rows from degree sort; pass 2 re-sorts with the fixed-lo count
    order = np.argsort(deg, kind="stable")
    core_of, local_of = deal(order)
    rows = _row_of(core_of, local_of, cfg)
    a = np.bincount(dst[rows[src] < BHI], minlength=N)
    order = np.lexsort((a, deg))
    core_of, local_of = deal(order)
    rows = _row_of(core_of, local_of, cfg)

    srow = rows[src]
    cat = np.where(srow < BHI, 0, np.where(srow < LO_WIN, 1, 2)).astype(np.int8)
    a = np.bincount(dst[cat == 0], minlength=N)
    f = np.bincount(dst[cat == 1], minlength=N)
    b = np.bincount(dst[cat == 2], minlength=N)
    assert np.all(a + f + b == deg)

    tile_of = local_of // P

    # per-tile substream depths (uniform across cores) via flex T-scan
    Ulo = np.zeros(TILES, np.int64)
    Uhi = np.zeros(TILES, np.int64)
    Tt = np.zeros(TILES, np.int64)
    for t in range(TILES):
        m = tile_of == t
        at, ft, bt = a[m], f[m], b[m]
        best = None
        lo_T = int(at.max()) if at.size else 0
        hi_T = int((at + ft).max()) if at.size else 0
        for T in range(lo_T, hi_T + 1):
            x = np.clip(T - at, 0, ft)
            lo = int((at + x).max())
            hi = int((bt + ft - x).max())
            if best is None or lo + hi < best[0]:
                best = (lo + hi, lo, hi, T)
        _, Ulo[t], Uhi[t], Tt[t] = best

    xflex = np.clip(Tt[tile_of] - a, 0, f)
    n_lo = a + xflex

    # per-edge slot assignment: order edges by (dst, category)
    eorder = np.lexsort((cat, dst))
    sd = dst[eorder]
    sval = srow[eorder]
    starts = np.zeros(N + 1, np.int64)
    np.cumsum(deg, out=starts[1:])
    posw = np.arange(E, dtype=np.int64) - starts[sd]
    is_lo_e = posw < n_lo[sd]
    slot = np.where(is_lo_e, posw, posw - n_lo[sd])

    cumlo = np.zeros(TILES + 1, np.int64)
    np.cumsum(Ulo, out=cumlo[1:])
    cumhi = np.zeros(TILES + 1, np.int64)
    np.cumsum(Uhi, out=cumhi[1:])
    LO_CH, HI_CH = int(cumlo[-1]), int(cumhi[-1])

    # dummy row: a pad row inside [BHI, min(LO_WIN, RTOT))
    n_real = N // C
    dummy = None
    dummy_core = None
    if n_real < LOCAL:
        for c in range(C):
            r0 = int(_row_of(c, n_real, cfg))
            if BHI <= r0 < min(LO_WIN, RTOT):
                dummy = r0
                dummy_core = c
                break
    assert dummy is not None, "no pad row available for the dummy entry"

    lane = local_of[sd] % P
    tl = tile_of[sd]
    cr = core_of[sd]

    lo_stream = np.full((C, LO_CH * P), dummy, np.int64)
    hi_stream = np.full((C, HI_CH * P), dummy - BHI, np.int64)
    ml = is_lo_e
    mh = ~is_lo_e
    lo_pos = (cumlo[tl[ml]] + slot[ml]) * P + lane[ml]
    hi_pos = (cumhi[tl[mh]] + slot[mh]) * P + lane[mh]
    lo_stream[cr[ml], lo_pos] = sval[ml]
    hi_stream[cr[mh], hi_pos] = sval[mh] - BHI
    # Pad lanes gather only dummy rows (q = 0); their denominator stays
    # finite via the always-present self-loop term q_self = exp(lrelu(0))
    # = 1 (pad x columns are zero), and their outputs are discarded.
    assert lo_stream.min() >= 0 and lo_stream.max() < min(LO_WIN, RTOT)
    if HI_CH:
        assert hi_stream.min() >= 0 and hi_stream.max() < 32768

    def wrap(sarr):
        # stream position i -> [i % 16, i // 16]; the 16-partition block is
        # replicated to all 8 GPSIMD core groups (128 partitions).
        L = sarr.shape[1]
        if L == 0:
            return np.zeros((C, 128, 0), np.int16)
        w = np.ascontiguousarray(
            sarr.reshape(C, L // 16, 16).transpose(0, 2, 1)
        ).astype(np.int16)
        return np.tile(w, (1, 8, 1))

    # batches of tiles with bounded chunk totals
    batches = []
    t0 = 0
    while t0 < TILES:
        t1 = t0
        tot = 0
        while t1 < TILES and tot + Ulo[t1] + Uhi[t1] <= cfg["CAP"]:
            tot += Ulo[t1] + Uhi[t1]
            t1 += 1
        assert t1 > t0, f"tile {t0} exceeds CAP alone ({Ulo[t0]}+{Uhi[t0]})"
        batches.append((t0, t1))
        t0 = t1

    return dict(
        core_of=core_of,
        local_of=local_of,
        Ulo=Ulo,
        Uhi=Uhi,
        cumlo=cumlo,
        cumhi=cumhi,
        LO_CH=LO_CH,
        HI_CH=HI_CH,
        batches=batches,
        lo_idx=wrap(lo_stream),
        hi_idx=wrap(hi_stream),
        dummy=dummy,
        E_pad=(LO_CH + HI_CH) * P,
    )


def build_program(cfg, sched):
    """Emit the (core-uniform) Bass program."""
    N, C = cfg["N"], cfg["C"]
    DIMS = cfg["DIMS"]
    TILES, LOCAL, RTOT, BHI = _derived(cfg)
    US, cums, CH = sched["US"], sched["cums"], sched["CH"]
    BASES = sched["BASES"]
    NS = len(BASES)
    batches = sched["batches"]
    CAP = cfg["CAP"]
    MAXU = int(max(Ulo[t] + Uhi[t] for t in range(TILES)))
    F_LAST = DIMS[3]

    nc = bacc.Bacc(
        "TRN2", target_bir_lowering=False, debug=False, num_devices=C,
        num_swdge_queues=4, dynamic_dma_scratch_size=24576,
    )

    # ---- I/O ----
    x_t_in = nc.dram_tensor("x_t", [P, LOCAL], F32, kind="ExternalInput")
    w_in = [
        nc.dram_tensor(f"wfull{l}", [DIMS[l], DIMS[l + 1] + 2], F32,
                       kind="ExternalInput")
        for l in range(3)
    ]
    bb_in = [
        nc.dram_tensor(f"bb{l}", [P, DIMS[l + 1]], F32, kind="ExternalInput")
        for l in range(3)
    ]
    idx_in = [
        nc.dram_tensor(f"idx{s}", [128, max(CH[s] * 8, 8)], I16,
                       kind="ExternalInput")
        for s in range(NS)
    ]
    dums_in = nc.dram_tensor("dums", [3, 256], BF16, kind="ExternalInput")
    out_d = nc.dram_tensor("out_local", [LOCAL, F_LAST], F32,
                           kind="ExternalOutput")

    ROWB = cfg["ROWB"]

    with tile.TileContext(nc) as tc:
        with tc.tile_pool(name="consts", bufs=1) as cp, \
             tc.tile_pool(name="dram", bufs=1, space="DRAM") as dp, \
             tc.tile_pool(name="work", bufs=3) as wp, \
             tc.tile_pool(name="small", bufs=4) as rp, \
             tc.tile_pool(name="psA", bufs=2, space="PSUM") as psA, \
             tc.tile_pool(name="psB", bufs=2, space="PSUM") as psB, \
             tc.tile_pool(name="psC", bufs=3, space="PSUM") as psC:

            # ---- constants ----
            ident32 = cp.tile([P, P], F32, tag="ident32")
            make_identity(nc, ident32)
            identbf = cp.tile([P, P], BF16, tag="identbf")
            nc.vector.tensor_copy(identbf[:, :], ident32[:, :])

            w_sb = []
            bb_sb = []
            for l in range(3):
                wt = cp.tile([DIMS[l], DIMS[l + 1] + 2], F32, tag=f"w{l}",
                             name=f"w_sb{l}")
                nc.sync.dma_start(wt[:, :], w_in[l][:, :])
                w_sb.append(wt)
                bt = cp.tile([P, DIMS[l + 1]], F32, tag=f"bb{l}",
                             name=f"bb_sb{l}")
                nc.sync.dma_start(bt[:, :], bb_in[l][:, :])
                bb_sb.append(bt)

            idx_sb = []
            for s in range(NS):
                it = cp.tile([128, max(CH[s] * 8, 8)], I16, tag=f"idx{s}")
                nc.sync.dma_start(it[:, :], idx_in[s][:, :])
                idx_sb.append(it)
            dums_sb = cp.tile([3, 256], BF16, tag="dums_sb")
            nc.sync.dma_start(dums_sb[:, :], dums_in[:, :])

            d_all = [
                cp.tile([P, TILES], F32, tag=f"dall{l}", name=f"d_all{l}")
                for l in range(3)
            ]
            nxt = [
                cp.tile([P, TILES * DIMS[l + 1]], F32, tag=f"nxt{l}",
                        name=f"nxt{l}")
                for l in range(2)
            ]

            slices = [
                dp.tile([LOCAL, ROWB[l]], BF16, tag=f"slice{l}",
                        name=f"slice{l}")
                for l in range(3)
            ]
            # NOTE: addr_space="Shared" crashes NRT under the axon/PJRT
            # runtime (NRT_EXEC_UNIT_UNRECOVERABLE); Local-space output
            # works (bass warns it is slower).
            tables = [
                dp.tile([RTOT, ROWB[l]], BF16, tag=f"table{l}",
                        name=f"table{l}")
                for l in range(3)
            ]

            rg = [list(range(C))]

            STAGE = int(os.environ.get("GAT_STAGE", "99"))
            NLAYERS = min(3, max(1, STAGE // 10 + 1)) if STAGE < 99 else 3
            SUB = STAGE % 10 if STAGE < 99 else 9

            for l in range(NLAYERS):
                fi, fo = DIMS[l], DIMS[l + 1]

                # ---------- node phase ----------
                for t in range(TILES):
                    if l == 0:
                        xT = wp.tile([P, P], F32, tag="xT")
                        nc.sync.dma_start(
                            xT[:, :], x_t_in[:, t * P:(t + 1) * P])
                        xT_ap = xT[:fi, :]
                    else:
                        xv = nxt[l - 1][:, t * fi:(t + 1) * fi]
                        xT_ps = psA.tile([fi, P], F32, tag="xT_ps")
                        nc.tensor.transpose(xT_ps[:, :], xv, ident32[:, :])
                        xT = wp.tile([fi, P], F32, tag="xT")
                        nc.scalar.copy(xT[:, :], xT_ps[:, :])
                        xT_ap = xT[:, :]

                    h_ps = psB.tile([P, fo + 2], F32, tag="h_ps")
                    nc.tensor.matmul(h_ps[:, :], lhsT=xT_ap, rhs=w_sb[l][:, :],
                                     start=True, stop=True)

                    nc.vector.tensor_copy(
                        d_all[l][:, t:t + 1], h_ps[:, fo + 1:fo + 2])

                    stg = wp.tile([P, ROWB[l]], BF16, tag=f"stg{l}")
                    nc.scalar.copy(stg[:, 0:fo], h_ps[:, 0:fo])
                    nc.vector.memset(stg[:, fo:fo + 1], 1.0)
                    nc.vector.memset(stg[:, fo + 1:fo + 2], 0.0)
                    nc.vector.tensor_copy(
                        stg[:, fo + 2:fo + 4].bitcast(F32),
                        h_ps[:, fo:fo + 1])
                    nc.vector.memset(stg[:, fo + 4:ROWB[l]], 0.0)
                    nc.sync.dma_start(
                        slices[l][t * P:(t + 1) * P, :], stg[:, :])

                if l == NLAYERS - 1 and SUB < 1:
                    continue
                # ---------- dummy-row patch + all-gather ----------
                # Every core overwrites its pad row `n_real` with
                # [h=0.., s=-1e30, one=0]; only core DUMMY_CORE's copy is ever
                # gathered (as the padding target), the rest are inert.
                n_real = N // C
                nc.sync.dma_start(
                    slices[l][n_real:n_real + 1, :],
                    dums_sb[l:l + 1, 0:ROWB[l]])
                nc.gpsimd.collective_compute(
                    "AllGather",
                    ALU.bypass,
                    replica_groups=rg,
                    ins=[slices[l][:, :].opt()],
                    outs=[tables[l][:, :].opt()],
                )

                # ---------- edge phase ----------
                if l == NLAYERS - 1 and SUB < 2:
                    continue
                with tc.tile_pool(name=f"mbuf{l}", bufs=2) as mp, \
                     tc.tile_pool(name=f"ebuf{l}", bufs=2) as ep:
                  for bi, (t0, t1) in enumerate(batches):
                    nlo = int(cumlo[t1] - cumlo[t0])
                    nhi = int(cumhi[t1] - cumhi[t0])
                    nch = nlo + nhi
                    mb_t = mp.tile([P, CAP, ROWB[l]], BF16, tag="mb")
                    if nlo:
                        nc.gpsimd.dma_gather(
                            out_ap=mb_t[:, 0:nlo, :],
                            in_ap=tables[l][:, :],
                            idxs_ap=lo_sb[:, int(cumlo[t0]) * 8:
                                          int(cumlo[t1]) * 8],
                            num_idxs=P * nlo,
                            num_idxs_reg=P * nlo,
                            elem_size=ROWB[l],
                            single_packet=False,
                            queue_num=(2 * bi) % 4,
                        )
                    if nhi:
                        nc.gpsimd.dma_gather(
                            out_ap=mb_t[:, nlo:nch, :],
                            in_ap=tables[l][BHI:RTOT, :],
                            idxs_ap=hi_sb[:, int(cumhi[t0]) * 8:
                                          int(cumhi[t1]) * 8],
                            num_idxs=P * nhi,
                            num_idxs_reg=P * nhi,
                            elem_size=ROWB[l],
                            single_packet=False,
                            queue_num=(2 * bi + 1) % 4,
                        )

                    if l == NLAYERS - 1 and SUB < 3:
                        continue

                    # per-(tile, substream) biased s extraction; zz/lre
                    # accumulate the whole batch in chunk order.
                    zzb = ep.tile([P, CAP], F32, tag="zzb")
                    lreb = ep.tile([P, CAP], F32, tag="lreb")
                    qb = ep.tile([P, CAP], BF16, tag="qb")
                    sqall = ep.tile([P, CAP * P], BF16, tag="sqall")
                    for t in range(t0, t1):
                        ulo = int(Ulo[t])
                        uhi = int(Uhi[t])
                        if ulo + uhi == 0:
                            continue
                        lob = int(cumlo[t] - cumlo[t0])
                        hib = nlo + int(cumhi[t] - cumhi[t0])
                        dcol = d_all[l][:, t:t + 1]
                        if ulo:
                            sview = mb_t[:, lob:lob + ulo,
                                         fo + 2:fo + 4].bitcast(F32)
                            nc.scalar.activation(
                                zzb[:, lob:lob + ulo], sview, AF.Identity,
                                bias=dcol)
                            nc.scalar.activation(
                                lreb[:, lob:lob + ulo], sview, AF.Relu,
                                bias=dcol)
                        if uhi:
                            sview = mb_t[:, hib:hib + uhi,
                                         fo + 2:fo + 4].bitcast(F32)
                            nc.scalar.activation(
                                zzb[:, hib:hib + uhi], sview, AF.Identity,
                                bias=dcol)
                            nc.scalar.activation(
                                lreb[:, hib:hib + uhi], sview, AF.Relu,
                                bias=dcol)

                    # lrelu(z) = a*(z + r*relu(z)), r=(1-a)/a; fold `a`
                    # into Exp's scale.  One op per batch.
                    a = cfg["NEG_SLOPE"]
                    nc.vector.scalar_tensor_tensor(
                        out=lreb[:, 0:nch], in0=lreb[:, 0:nch],
                        scalar=(1.0 - a) / a, in1=zzb[:, 0:nch],
                        op0=ALU.mult, op1=ALU.add)
                    nc.scalar.activation(
                        qb[:, 0:nch], lreb[:, 0:nch], AF.Exp, scale=a)

                    # batched diag build: sqall[p, c*P+j] =
                    #   identbf[p, j] * qb[p, c]   (stride-0 broadcasts)
                    vi = identbf[:, :]
                    vq = qb[:, :]
                    vo = sqall[:, :]
                    nc.vector.tensor_tensor(
                        out=bass.AP(vo.tensor, vo.offset,
                                    [list(vo.ap[0]), [P, nch], [1, P]]),
                        in0=bass.AP(vi.tensor, vi.offset,
                                    [list(vi.ap[0]), [0, nch], [1, P]]),
                        in1=bass.AP(vq.tensor, vq.offset,
                                    [list(vq.ap[0]), [1, nch], [0, P]]),
                        op=ALU.mult)

                    for t in range(t0, t1):
                        ulo = int(Ulo[t])
                        uhi = int(Uhi[t])
                        U = ulo + uhi
                        if U == 0:
                            continue
                        lob = int(cumlo[t] - cumlo[t0])
                        hib = nlo + int(cumhi[t] - cumhi[t0])

                        if l == NLAYERS - 1 and SUB < 4:
                            continue
                        acc = psC.tile([P, fo + 1], F32, tag="acc")
                        for u in range(U):
                            ch = (lob + u) if u < ulo else (hib + (u - ulo))
                            nc.tensor.matmul(
                                acc[:, :],
                                lhsT=sqall[:, ch * P:(ch + 1) * P],
                                rhs=mb_t[:, ch, 0:fo + 1],
                                start=(u == 0), stop=(u == U - 1))

                        rc = rp.tile([P, 1], F32, tag="rc")
                        nc.vector.reciprocal(rc[:, :], acc[:, fo:fo + 1])
                        o_sb = wp.tile([P, fo], F32, tag="o_sb")
                        nc.vector.scalar_tensor_tensor(
                            out=o_sb[:, :], in0=acc[:, 0:fo], scalar=rc[:, :],
                            in1=bb_sb[l][:, :], op0=ALU.mult, op1=ALU.add)

                        if l < 2:
                            # SiLU via the exp table: x / (1 + exp(-x))
                            ex = wp.tile([P, fo], F32, tag="silu_e")
                            nc.scalar.activation(
                                ex[:, :], o_sb[:, :], AF.Exp, scale=-1.0)
                            nc.vector.tensor_scalar(
                                out=ex[:, :], in0=ex[:, :], scalar1=1.0,
                                scalar2=None, op0=ALU.add)
                            nc.vector.reciprocal(ex[:, :], ex[:, :])
                            nc.vector.tensor_tensor(
                                out=nxt[l][:, t * fo:(t + 1) * fo],
                                in0=o_sb[:, :], in1=ex[:, :], op=ALU.mult)
                        else:
                            mneg = rp.tile([P, 1], F32, tag="mneg")
                            nc.vector.tensor_reduce(
                                mneg[:, :], o_sb[:, :], axis=AXL.X,
                                op=ALU.max, negate=True)
                            ex2 = wp.tile([P, fo], F32, tag="ls_e")
                            se = rp.tile([P, 1], F32, tag="se")
                            nc.scalar.activation(
                                ex2[:, :], o_sb[:, :], AF.Exp, bias=mneg[:, :],
                                accum_out=se[:, :])
                            lse = rp.tile([P, 1], F32, tag="lse")
                            nc.scalar.activation(lse[:, :], se[:, :], AF.Ln)
                            fin = wp.tile([P, fo], F32, tag="fin")
                            nc.vector.tensor_scalar(
                                out=fin[:, :], in0=o_sb[:, :],
                                scalar1=mneg[:, :], scalar2=lse[:, :],
                                op0=ALU.add, op1=ALU.subtract)
                            nc.sync.dma_start(
                                out_d[t * P:(t + 1) * P, :], fin[:, :])

    nc.compile()
    return nc


def make_inputs(x, weights, cfg, sched):
    """Build the per-core in_maps."""
    N, C = cfg["N"], cfg["C"]
    TILES, LOCAL, RTOT, BHI = _derived(cfg)
    DIMS = cfg["DIMS"]
    core_of, local_of = sched["core_of"], sched["local_of"]

    x = np.asarray(x, np.float32)
    common = {}
    for l in range(3):
        W = np.asarray(weights[f"W{l}"], np.float64)
        a_s = np.asarray(weights[f"a_src{l}"], np.float64)
        a_d = np.asarray(weights[f"a_dst{l}"], np.float64)
        wfull = np.concatenate(
            [W, (W @ a_s)[:, None], (W @ a_d)[:, None]], axis=1)
        common[f"wfull{l}"] = np.ascontiguousarray(wfull, dtype=np.float32)
        b = np.asarray(weights[f"b{l}"], np.float32)
        common[f"bb{l}"] = np.ascontiguousarray(
            np.broadcast_to(b, (P, DIMS[l + 1])), dtype=np.float32)
    dums = np.zeros((3, 256), np.uint16)
    sneg = np.array([S_NEG], np.float32).view(np.uint16)
    for l in range(3):
        fo = DIMS[l + 1]
        # one = 1.0 keeps the patched pad row's SELF-loop denominator at 1
        # (gathered dummy slots still contribute 0: their q = exp(-inf));
        # without it lane n_real%P of the last tile divides by zero and the
        # NaN poisons real lanes via 0*NaN in the diag matmuls.
        dums[l, fo] = 0x3F80  # bf16 1.0
        dums[l, fo + 2:fo + 4] = sneg
    common["dums"] = dums.view(ml_dtypes.bfloat16).copy()

    in_maps = []
    for c in range(C):
        m = dict(common)
        nodes = np.where(core_of == c)[0]
        xt = np.zeros((P, LOCAL), np.float32)
        xt[:, local_of[nodes]] = x[nodes].T
        m["x_t"] = xt
        for s in range(len(sched["BASES"])):
            m[f"idx{s}"] = np.ascontiguousarray(
                sched["idx"][s][c] if sched["CH"][s] else
                np.zeros((128, 8), np.int16))
        in_maps.append(m)
    return in_maps


LAST_EXEC_NS = None
LAST_RESULTS = None


def run(inputs, cfg=None, trace=False):
    global LAST_EXEC_NS, LAST_RESULTS
    cfg = cfg or default_cfg()
    N, C = cfg["N"], cfg["C"]
    TILES, LOCAL, RTOT, BHI = _derived(cfg)

    sched = preprocess(np.asarray(inputs["edge_index"]), cfg)
    nc = build_program(cfg, sched)
    in_maps = make_inputs(inputs["x"], inputs, cfg, sched)

    res = run_bass_kernel_spmd(
        nc, in_maps, core_ids=list(range(C)), trace=trace,
        stitch_traces=trace,
    )
    LAST_EXEC_NS = res.exec_time_ns
    LAST_RESULTS = res

    F_LAST = cfg["DIMS"][3]
    out = np.empty((N, F_LAST), np.float32)
    core_of, local_of = sched["core_of"], sched["local_of"]
    for c in range(C):
        nodes = np.where(core_of == c)[0]
        out[nodes] = res.results[c]["out_local"][local_of[nodes]]
    return out


def kernel(**inputs):
    return run(inputs, trace=bool(int(os.environ.get("GAT_TRACE", "0"))))



# revision 26
# speedup vs baseline: 1.7345x; 1.0014x over previous
"""3-layer GAT on Trainium2, 8 NeuronCores.

Strategy (graph/data parallel, dst-sharded).  The kernel is bound by
dma_gather's per-row DMA descriptors (HBM random-read latency), so the
design minimizes gathered rows and keeps all 4 SWDGE queue rings loaded:

  - Destination nodes are dealt round-robin (by degree rank) across 8
    cores; each core owns LOCAL=6272 node slots (6250 real + 22 pad).
  - Per layer: each core computes h = x @ W for its nodes (plus attention
    scalars s = h.a_src, d = h.a_dst via host-precomputed W@a columns),
    stages a bf16 row [h | 1 | 0 | s_f32] per node (512B rows for
    fo=128, 256B otherwise), and publishes it via FOUR partial
    AllGathers (chunked by node-tile range) that overlap the previous
    layer's edge phase; next-layer node tiles are emitted inline in the
    edge loop so the engines actually interleave them.
  - Edge layout is slot-major with lane = destination: chunk = one slot
    for the 128 destinations of a tile, so d is a per-partition bias,
    the segment softmax is a per-partition row op, and aggregation is a
    per-chunk diag(q) matmul accumulated in PSUM (denominator = the
    constant-1 row column).  All per-chunk diag matrices of a batch are
    built in ONE DVE tensor_tensor using stride-0 broadcast APs.
  - Self-loop edges are never gathered: their rows are core-local, added
    per tile as one extra matmul (diag(q_self) x [h | 1] re-read densely
    from the slice).
  - dma_gather indices are int16, so the 50176-row table is addressed
    through THREE overlapping 32768-row windows (bases 0, BHI/2, BHI =
    plain in_ap offsets); each edge is assigned to a window stream by a
    per-tile balance optimizer, cutting slot padding to ~25% over the
    single-stream floor.  Gathers are split in half and striped over the
    4 SWDGE queues: SDMA engines round-robin rings at packet granularity,
    which overlaps the HBM read latency of different rings.
  - Softmax max-subtraction is skipped (max |e| ~ 9 here, exp is safe in
    fp32); padding slots gather a dummy pad row whose s = -1e30 (q = 0)
    and one = 1.0 (keeps the patched row's self-loop denominator at 1).
"""

import os
import sys

for _p in ("/opt/trn_rl_repo", "/opt/pypackages"):
    if os.path.isdir(_p) and _p not in sys.path:
        sys.path.insert(0, _p)

import ml_dtypes
import numpy as np

import concourse.bacc as bacc
import concourse.bass as bass
import concourse.mybir as mybir
import concourse.tile as tile
from concourse.bass_utils import run_bass_kernel_spmd
from concourse.masks import make_identity

F32 = mybir.dt.float32
BF16 = mybir.dt.bfloat16
I16 = mybir.dt.int16
AF = mybir.ActivationFunctionType
ALU = mybir.AluOpType
AXL = mybir.AxisListType

P = 128
S_NEG = -1e30


def default_cfg():
    return dict(
        N=50000,
        C=8,
        DIMS=(128, 128, 64, 40),
        LO_WIN=32768,
        CAP=48,  # max gathered chunks per batch (SBUF budget)
        NEG_SLOPE=0.2,
        ROWB=(256, 128, 128),  # bf16 slots per table row, per layer
    )


def _derived(cfg):
    N, C = cfg["N"], cfg["C"]
    assert N % C == 0
    tiles = (N // C + P - 1) // P
    local = tiles * P
    rtot = C * local
    bhi = max(0, rtot - cfg["LO_WIN"])
    return tiles, local, rtot, bhi


# AllGather chunking: the table is built by 4 partial AllGathers over
# node-tile ranges so each chunk can launch as soon as its node tiles are
# staged (hiding the collective under the previous layer's edge phase).
# Each chunk's output block is [core0-rows | core1-rows | ...]; the
# pad-bearing chunk (last tile range) is placed SECOND in table order so
# some core's pad row lands in the lo/hi index overlap [BHI, LO_WIN) and
# can serve as the dummy gather target.
AG_BOUNDS = (0, 13, 26, 38, 49)  # node-tile range boundaries
AG_TORD = (0, 3, 1, 2)  # range index -> position in table order


def _ag_blocks(cfg):
    """Returns [(lo_local, hi_local, table_base)] indexed by range id."""
    C = cfg["C"]
    ranges = [
        (AG_BOUNDS[i] * P, AG_BOUNDS[i + 1] * P) for i in range(4)
    ]
    base = 0
    tb = {}
    for ri in AG_TORD:
        lo, hi = ranges[ri]
        tb[ri] = base
        base += (hi - lo) * C
    return [(lo, hi, tb[ri]) for ri, (lo, hi) in enumerate(ranges)]


def _row_of(core, local, cfg):
    """Table row of (core, local) under the chunked-AllGather layout."""
    core, local = np.broadcast_arrays(np.asarray(core), np.asarray(local))
    row = np.empty(local.shape, np.int64)
    for lo, hi, tbase in _ag_blocks(cfg):
        m = (local >= lo) & (local < hi)
        row[m] = tbase + core[m] * (hi - lo) + (local[m] - lo)
    return row


def preprocess(edge_index, cfg):
    """Host-side graph scheduling.  Returns a dict of per-core arrays and
    the (core-uniform) tile schedule."""
    N, C = cfg["N"], cfg["C"]
    TILES, LOCAL, RTOT, BHI = _derived(cfg)
    LO_WIN = cfg["LO_WIN"]

    # Self-loop edges (the reference appends one per node) are NOT put in
    # the gather streams: their source rows are core-local, so the kernel
    # adds the q_self * [h | 1] contribution with one extra per-tile
    # matmul instead of gathering 50k rows.
    src = np.asarray(edge_index[0], dtype=np.int64)
    dst = np.asarray(edge_index[1], dtype=np.int64)
    E = src.shape[0]
    deg = np.bincount(dst, minlength=N)

    def deal(order):
        core_of = np.empty(N, np.int64)
        local_of = np.empty(N, np.int64)
        r = np.arange(N)
        core_of[order] = r % C
        local_of[order] = r // C
        return core_of, local_of

    # pass 1: rows from degree sort; pass 2 re-sorts with the fixed-lo count
    order = np.argsort(deg, kind="stable")
    core_of, local_of = deal(order)
    rows = _row_of(core_of, local_of, cfg)
    a = np.bincount(dst[rows[src] < BHI], minlength=N)
    order = np.lexsort((a, deg))
    core_of, local_of = deal(order)
    rows = _row_of(core_of, local_of, cfg)

    srow = rows[src]
    cat = np.where(srow < BHI, 0, np.where(srow < LO_WIN, 1, 2)).astype(np.int8)
    a = np.bincount(dst[cat == 0], minlength=N)
    f = np.bincount(dst[cat == 1], minlength=N)
    b = np.bincount(dst[cat == 2], minlength=N)
    assert np.all(a + f + b == deg)

    tile_of = local_of // P

    # per-tile substream depths (uniform across cores) via flex T-scan
    Ulo = np.zeros(TILES, np.int64)
    Uhi = np.zeros(TILES, np.int64)
    Tt = np.zeros(TILES, np.int64)
    for t in range(TILES):
        m = tile_of == t
        at, ft, bt = a[m], f[m], b[m]
        best = None
        lo_T = int(at.max()) if at.size else 0
        hi_T = int((at + ft).max()) if at.size else 0
        for T in range(lo_T, hi_T + 1):
            x = np.clip(T - at, 0, ft)
            lo = int((at + x).max())
            hi = int((bt + ft - x).max())
            if best is None or lo + hi < best[0]:
                best = (lo + hi, lo, hi, T)
        _, Ulo[t], Uhi[t], Tt[t] = best

    xflex = np.clip(Tt[tile_of] - a, 0, f)
    n_lo = a + xflex

    # per-edge slot assignment: order edges by (dst, category)
    eorder = np.lexsort((cat, dst))
    sd = dst[eorder]
    sval = srow[eorder]
    starts = np.zeros(N + 1, np.int64)
    np.cumsum(deg, out=starts[1:])
    posw = np.arange(E, dtype=np.int64) - starts[sd]
    is_lo_e = posw < n_lo[sd]
    slot = np.where(is_lo_e, posw, posw - n_lo[sd])

    cumlo = np.zeros(TILES + 1, np.int64)
    np.cumsum(Ulo, out=cumlo[1:])
    cumhi = np.zeros(TILES + 1, np.int64)
    np.cumsum(Uhi, out=cumhi[1:])
    LO_CH, HI_CH = int(cumlo[-1]), int(cumhi[-1])

    # dummy row: a pad row inside [BHI, min(LO_WIN, RTOT))
    n_real = N // C
    dummy = None
    dummy_core = None
    if n_real < LOCAL:
        for c in range(C):
            r0 = int(_row_of(c, n_real, cfg))
            if BHI <= r0 < min(LO_WIN, RTOT):
                dummy = r0
                dummy_core = c
                break
    assert dummy is not None, "no pad row available for the dummy entry"

    lane = local_of[sd] % P
    tl = tile_of[sd]
    cr = core_of[sd]

    lo_stream = np.full((C, LO_CH * P), dummy, np.int64)
    hi_stream = np.full((C, HI_CH * P), dummy - BHI, np.int64)
    ml = is_lo_e
    mh = ~is_lo_e
    lo_pos = (cumlo[tl[ml]] + slot[ml]) * P + lane[ml]
    hi_pos = (cumhi[tl[mh]] + slot[mh]) * P + lane[mh]
    lo_stream[cr[ml], lo_pos] = sval[ml]
    hi_stream[cr[mh], hi_pos] = sval[mh] - BHI
    # Pad lanes gather only dummy rows (q = 0); their denominator stays
    # finite via the always-present self-loop term q_self = exp(lrelu(0))
    # = 1 (pad x columns are zero), and their outputs are discarded.
    assert lo_stream.min() >= 0 and lo_stream.max() < min(LO_WIN, RTOT)
    if HI_CH:
        assert hi_stream.min() >= 0 and hi_stream.max() < 32768

    def wrap(sarr):
        # stream position i -> [i % 16, i // 16]; the 16-partition block is
        # replicated to all 8 GPSIMD core groups (128 partitions).
        L = sarr.shape[1]
        if L == 0:
            return np.zeros((C, 128, 0), np.int16)
        w = np.ascontiguousarray(
            sarr.reshape(C, L // 16, 16).transpose(0, 2, 1)
        ).astype(np.int16)
        return np.tile(w, (1, 8, 1))

    # batches of tiles with bounded chunk totals
    batches = []
    t0 = 0
    while t0 < TILES:
        t1 = t0
        tot = 0
        while t1 < TILES and tot + Ulo[t1] + Uhi[t1] <= cfg["CAP"]:
            tot += Ulo[t1] + Uhi[t1]
            t1 += 1
        assert t1 > t0, f"tile {t0} exceeds CAP alone ({Ulo[t0]}+{Uhi[t0]})"
        batches.append((t0, t1))
        t0 = t1

    return dict(
        core_of=core_of,
        local_of=local_of,
        Ulo=Ulo,
        Uhi=Uhi,
        cumlo=cumlo,
        cumhi=cumhi,
        LO_CH=LO_CH,
        HI_CH=HI_CH,
        batches=batches,
        lo_idx=wrap(lo_stream),
        hi_idx=wrap(hi_stream),
        dummy=dummy,
        E_pad=(LO_CH + HI_CH) * P,
    )


def build_program(cfg, sched):
    """Emit the (core-uniform) Bass program."""
    N, C = cfg["N"], cfg["C"]
    DIMS = cfg["DIMS"]
    TILES, LOCAL, RTOT, BHI = _derived(cfg)
    US, cums, CH = sched["US"], sched["cums"], sched["CH"]
    BASES = sched["BASES"]
    NS = len(BASES)
    batches = sched["batches"]
    CAP = cfg["CAP"]
    MAXU = int(max(Ulo[t] + Uhi[t] for t in range(TILES)))
    F_LAST = DIMS[3]

    nc = bacc.Bacc(
        "TRN2", target_bir_lowering=False, debug=False, num_devices=C,
        num_swdge_queues=4, dynamic_dma_scratch_size=24576,
    )

    # ---- I/O ----
    x_t_in = nc.dram_tensor("x_t", [P, LOCAL], F32, kind="ExternalInput")
    w_in = [
        nc.dram_tensor(f"wfull{l}", [DIMS[l], DIMS[l + 1] + 2], F32,
                       kind="ExternalInput")
        for l in range(3)
    ]
    bb_in = [
        nc.dram_tensor(f"bb{l}", [P, DIMS[l + 1]], F32, kind="ExternalInput")
        for l in range(3)
    ]
    idx_in = [
        nc.dram_tensor(f"idx{s}", [128, max(CH[s] * 8, 8)], I16,
                       kind="ExternalInput")
        for s in range(NS)
    ]
    dums_in = nc.dram_tensor("dums", [3, 256], BF16, kind="ExternalInput")
    out_d = nc.dram_tensor("out_local", [LOCAL, F_LAST], F32,
                           kind="ExternalOutput")

    ROWB = cfg["ROWB"]

    with tile.TileContext(nc) as tc:
        with tc.tile_pool(name="consts", bufs=1) as cp, \
             tc.tile_pool(name="dram", bufs=1, space="DRAM") as dp, \
             tc.tile_pool(name="work", bufs=3) as wp, \
             tc.tile_pool(name="small", bufs=4) as rp, \
             tc.tile_pool(name="psA", bufs=2, space="PSUM") as psA, \
             tc.tile_pool(name="psB", bufs=2, space="PSUM") as psB, \
             tc.tile_pool(name="psC", bufs=3, space="PSUM") as psC:

            # ---- constants ----
            ident32 = cp.tile([P, P], F32, tag="ident32")
            make_identity(nc, ident32)
            identbf = cp.tile([P, P], BF16, tag="identbf")
            nc.vector.tensor_copy(identbf[:, :], ident32[:, :])

            w_sb = []
            bb_sb = []
            for l in range(3):
                wt = cp.tile([DIMS[l], DIMS[l + 1] + 2], F32, tag=f"w{l}",
                             name=f"w_sb{l}")
                nc.sync.dma_start(wt[:, :], w_in[l][:, :])
                w_sb.append(wt)
                bt = cp.tile([P, DIMS[l + 1]], F32, tag=f"bb{l}",
                             name=f"bb_sb{l}")
                nc.sync.dma_start(bt[:, :], bb_in[l][:, :])
                bb_sb.append(bt)

            idx_sb = []
            for s in range(NS):
                it = cp.tile([128, max(CH[s] * 8, 8)], I16, tag=f"idx{s}")
                nc.sync.dma_start(it[:, :], idx_in[s][:, :])
                idx_sb.append(it)
            dums_sb = cp.tile([3, 256], BF16, tag="dums_sb")
            nc.sync.dma_start(dums_sb[:, :], dums_in[:, :])

            d_all = [
                cp.tile([P, TILES], F32, tag=f"dall{l}", name=f"d_all{l}")
                for l in range(3)
            ]
            nxt = [
                cp.tile([P, TILES * DIMS[l + 1]], F32, tag=f"nxt{l}",
                        name=f"nxt{l}")
                for l in range(2)
            ]

            slices = [
                dp.tile([LOCAL, ROWB[l]], BF16, tag=f"slice{l}",
                        name=f"slice{l}")
                for l in range(3)
            ]
            # NOTE: addr_space="Shared" crashes NRT under the axon/PJRT
            # runtime (NRT_EXEC_UNIT_UNRECOVERABLE); Local-space output
            # works (bass warns it is slower).
            tables = [
                dp.tile([RTOT, ROWB[l]], BF16, tag=f"table{l}",
                        name=f"table{l}")
                for l in range(3)
            ]

            rg = [list(range(C))]

            STAGE = int(os.environ.get("GAT_STAGE", "99"))
            NLAYERS = min(3, max(1, STAGE // 10 + 1)) if STAGE < 99 else 3
            SUB = STAGE % 10 if STAGE < 99 else 9

            for l in range(NLAYERS):
                fi, fo = DIMS[l], DIMS[l + 1]

                # ---------- node phase ----------
                for t in range(TILES):
                    if l == 0:
                        xT = wp.tile([P, P], F32, tag="xT")
                        nc.sync.dma_start(
                            xT[:, :], x_t_in[:, t * P:(t + 1) * P])
                        xT_ap = xT[:fi, :]
                    else:
                        xv = nxt[l - 1][:, t * fi:(t + 1) * fi]
                        xT_ps = psA.tile([fi, P], F32, tag="xT_ps")
                        nc.tensor.transpose(xT_ps[:, :], xv, ident32[:, :])
                        xT = wp.tile([fi, P], F32, tag="xT")
                        nc.scalar.copy(xT[:, :], xT_ps[:, :])
                        xT_ap = xT[:, :]

                    h_ps = psB.tile([P, fo + 2], F32, tag="h_ps")
                    nc.tensor.matmul(h_ps[:, :], lhsT=xT_ap, rhs=w_sb[l][:, :],
                                     start=True, stop=True)

                    nc.vector.tensor_copy(
                        d_all[l][:, t:t + 1], h_ps[:, fo + 1:fo + 2])

                    stg = wp.tile([P, ROWB[l]], BF16, tag=f"stg{l}")
                    nc.scalar.copy(stg[:, 0:fo], h_ps[:, 0:fo])
                    nc.vector.memset(stg[:, fo:fo + 1], 1.0)
                    nc.vector.memset(stg[:, fo + 1:fo + 2], 0.0)
                    nc.vector.tensor_copy(
                        stg[:, fo + 2:fo + 4].bitcast(F32),
                        h_ps[:, fo:fo + 1])
                    nc.vector.memset(stg[:, fo + 4:ROWB[l]], 0.0)
                    nc.sync.dma_start(
                        slices[l][t * P:(t + 1) * P, :], stg[:, :])

                if l == NLAYERS - 1 and SUB < 1:
                    continue
                # ---------- dummy-row patch + all-gather ----------
                # Every core overwrites its pad row `n_real` with
                # [h=0.., s=-1e30, one=0]; only core DUMMY_CORE's copy is ever
                # gathered (as the padding target), the rest are inert.
                n_real = N // C
                nc.sync.dma_start(
                    slices[l][n_real:n_real + 1, :],
                    dums_sb[l:l + 1, 0:ROWB[l]])
                nc.gpsimd.collective_compute(
                    "AllGather",
                    ALU.bypass,
                    replica_groups=rg,
                    ins=[slices[l][:, :].opt()],
                    outs=[tables[l][:, :].opt()],
                )

                # ---------- edge phase ----------
                if l == NLAYERS - 1 and SUB < 2:
                    continue
                with tc.tile_pool(name=f"mbuf{l}", bufs=3) as mp, \
                     tc.tile_pool(name=f"ebuf{l}", bufs=2) as ep:
                  for bi, (t0, t1) in enumerate(batches):
                    nlo = int(cumlo[t1] - cumlo[t0])
                    nhi = int(cumhi[t1] - cumhi[t0])
                    nch = nlo + nhi
                    mb_t = mp.tile([P, CAP, ROWB[l]], BF16, tag="mb")
                    if nlo:
                        nc.gpsimd.dma_gather(
                            out_ap=mb_t[:, 0:nlo, :],
                            in_ap=tables[l][:, :],
                            idxs_ap=lo_sb[:, int(cumlo[t0]) * 8:
                                          int(cumlo[t1]) * 8],
                            num_idxs=P * nlo,
                            num_idxs_reg=P * nlo,
                            elem_size=ROWB[l],
                            single_packet=False,
                            queue_num=(2 * bi) % 4,
                        )
                    if nhi:
                        nc.gpsimd.dma_gather(
                            out_ap=mb_t[:, nlo:nch, :],
                            in_ap=tables[l][BHI:RTOT, :],
                            idxs_ap=hi_sb[:, int(cumhi[t0]) * 8:
                                          int(cumhi[t1]) * 8],
                            num_idxs=P * nhi,
                            num_idxs_reg=P * nhi,
                            elem_size=ROWB[l],
                            single_packet=False,
                            queue_num=(2 * bi + 1) % 4,
                        )

                    if l == NLAYERS - 1 and SUB < 3:
                        continue

                    # per-(tile, substream) biased s extraction; zz/lre
                    # accumulate the whole batch in chunk order.
                    zzb = ep.tile([P, CAP], F32, tag="zzb")
                    lreb = ep.tile([P, CAP], F32, tag="lreb")
                    qb = ep.tile([P, CAP], BF16, tag="qb")
                    sqall = ep.tile([P, CAP * P], BF16, tag="sqall")
                    for t in range(t0, t1):
                        ulo = int(Ulo[t])
                        uhi = int(Uhi[t])
                        if ulo + uhi == 0:
                            continue
                        lob = int(cumlo[t] - cumlo[t0])
                        hib = nlo + int(cumhi[t] - cumhi[t0])
                        dcol = d_all[l][:, t:t + 1]
                        if ulo:
                            sview = mb_t[:, lob:lob + ulo,
                                         fo + 2:fo + 4].bitcast(F32)
                            nc.scalar.activation(
                                zzb[:, lob:lob + ulo], sview, AF.Identity,
                                bias=dcol)
                            nc.scalar.activation(
                                lreb[:, lob:lob + ulo], sview, AF.Relu,
                                bias=dcol)
                        if uhi:
                            sview = mb_t[:, hib:hib + uhi,
                                         fo + 2:fo + 4].bitcast(F32)
                            nc.scalar.activation(
                                zzb[:, hib:hib + uhi], sview, AF.Identity,
                                bias=dcol)
                            nc.scalar.activation(
                                lreb[:, hib:hib + uhi], sview, AF.Relu,
                                bias=dcol)

                    # lrelu(z) = a*(z + r*relu(z)), r=(1-a)/a; fold `a`
                    # into Exp's scale.  One op per batch.
                    a = cfg["NEG_SLOPE"]
                    nc.vector.scalar_tensor_tensor(
                        out=lreb[:, 0:nch], in0=lreb[:, 0:nch],
                        scalar=(1.0 - a) / a, in1=zzb[:, 0:nch],
                        op0=ALU.mult, op1=ALU.add)
                    nc.scalar.activation(
                        qb[:, 0:nch], lreb[:, 0:nch], AF.Exp, scale=a)

                    # batched diag build: sqall[p, c*P+j] =
                    #   identbf[p, j] * qb[p, c]   (stride-0 broadcasts)
                    vi = identbf[:, :]
                    vq = qb[:, :]
                    vo = sqall[:, :]
                    nc.vector.tensor_tensor(
                        out=bass.AP(vo.tensor, vo.offset,
                                    [list(vo.ap[0]), [P, nch], [1, P]]),
                        in0=bass.AP(vi.tensor, vi.offset,
                                    [list(vi.ap[0]), [0, nch], [1, P]]),
                        in1=bass.AP(vq.tensor, vq.offset,
                                    [list(vq.ap[0]), [1, nch], [0, P]]),
                        op=ALU.mult)

                    for t in range(t0, t1):
                        ulo = int(Ulo[t])
                        uhi = int(Uhi[t])
                        U = ulo + uhi
                        if U == 0:
                            continue
                        lob = int(cumlo[t] - cumlo[t0])
                        hib = nlo + int(cumhi[t] - cumhi[t0])

                        if l == NLAYERS - 1 and SUB < 4:
                            continue
                        acc = psC.tile([P, fo + 1], F32, tag="acc")
                        for u in range(U):
                            ch = (lob + u) if u < ulo else (hib + (u - ulo))
                            nc.tensor.matmul(
                                acc[:, :],
                                lhsT=sqall[:, ch * P:(ch + 1) * P],
                                rhs=mb_t[:, ch, 0:fo + 1],
                                start=(u == 0), stop=(u == U - 1))

                        rc = rp.tile([P, 1], F32, tag="rc")
                        nc.vector.reciprocal(rc[:, :], acc[:, fo:fo + 1])
                        o_sb = wp.tile([P, fo], F32, tag="o_sb")
                        nc.vector.scalar_tensor_tensor(
                            out=o_sb[:, :], in0=acc[:, 0:fo], scalar=rc[:, :],
                            in1=bb_sb[l][:, :], op0=ALU.mult, op1=ALU.add)

                        if l < 2:
                            # SiLU via the exp table: x / (1 + exp(-x))
                            ex = wp.tile([P, fo], F32, tag="silu_e")
                            nc.scalar.activation(
                                ex[:, :], o_sb[:, :], AF.Exp, scale=-1.0)
                            nc.vector.tensor_scalar(
                                out=ex[:, :], in0=ex[:, :], scalar1=1.0,
                                scalar2=None, op0=ALU.add)
                            nc.vector.reciprocal(ex[:, :], ex[:, :])
                            nc.vector.tensor_tensor(
                                out=nxt[l][:, t * fo:(t + 1) * fo],
                                in0=o_sb[:, :], in1=ex[:, :], op=ALU.mult)
                        else:
                            mneg = rp.tile([P, 1], F32, tag="mneg")
                            nc.vector.tensor_reduce(
                                mneg[:, :], o_sb[:, :], axis=AXL.X,
                                op=ALU.max, negate=True)
                            ex2 = wp.tile([P, fo], F32, tag="ls_e")
                            se = rp.tile([P, 1], F32, tag="se")
                            nc.scalar.activation(
                                ex2[:, :], o_sb[:, :], AF.Exp, bias=mneg[:, :],
                                accum_out=se[:, :])
                            lse = rp.tile([P, 1], F32, tag="lse")
                            nc.scalar.activation(lse[:, :], se[:, :], AF.Ln)
                            fin = wp.tile([P, fo], F32, tag="fin")
                            nc.vector.tensor_scalar(
                                out=fin[:, :], in0=o_sb[:, :],
                                scalar1=mneg[:, :], scalar2=lse[:, :],
                                op0=ALU.add, op1=ALU.subtract)
                            nc.sync.dma_start(
                                out_d[t * P:(t + 1) * P, :], fin[:, :])

    nc.compile()
    return nc


def make_inputs(x, weights, cfg, sched):
    """Build the per-core in_maps."""
    N, C = cfg["N"], cfg["C"]
    TILES, LOCAL, RTOT, BHI = _derived(cfg)
    DIMS = cfg["DIMS"]
    core_of, local_of = sched["core_of"], sched["local_of"]

    x = np.asarray(x, np.float32)
    common = {}
    for l in range(3):
        W = np.asarray(weights[f"W{l}"], np.float64)
        a_s = np.asarray(weights[f"a_src{l}"], np.float64)
        a_d = np.asarray(weights[f"a_dst{l}"], np.float64)
        wfull = np.concatenate(
            [W, (W @ a_s)[:, None], (W @ a_d)[:, None]], axis=1)
        common[f"wfull{l}"] = np.ascontiguousarray(wfull, dtype=np.float32)
        b = np.asarray(weights[f"b{l}"], np.float32)
        common[f"bb{l}"] = np.ascontiguousarray(
            np.broadcast_to(b, (P, DIMS[l + 1])), dtype=np.float32)
    dums = np.zeros((3, 256), np.uint16)
    sneg = np.array([S_NEG], np.float32).view(np.uint16)
    for l in range(3):
        fo = DIMS[l + 1]
        # one = 1.0 keeps the patched pad row's SELF-loop denominator at 1
        # (gathered dummy slots still contribute 0: their q = exp(-inf));
        # without it lane n_real%P of the last tile divides by zero and the
        # NaN poisons real lanes via 0*NaN in the diag matmuls.
        dums[l, fo] = 0x3F80  # bf16 1.0
        dums[l, fo + 2:fo + 4] = sneg
    common["dums"] = dums.view(ml_dtypes.bfloat16).copy()

    in_maps = []
    for c in range(C):
        m = dict(common)
        nodes = np.where(core_of == c)[0]
        xt = np.zeros((P, LOCAL), np.float32)
        xt[:, local_of[nodes]] = x[nodes].T
        m["x_t"] = xt
        for s in range(len(sched["BASES"])):
            m[f"idx{s}"] = np.ascontiguousarray(
                sched["idx"][s][c] if sched["CH"][s] else
                np.zeros((128, 8), np.int16))
        in_maps.append(m)
    return in_maps


LAST_EXEC_NS = None
LAST_RESULTS = None


def run(inputs, cfg=None, trace=False):
    global LAST_EXEC_NS, LAST_RESULTS
    cfg = cfg or default_cfg()
    N, C = cfg["N"], cfg["C"]
    TILES, LOCAL, RTOT, BHI = _derived(cfg)

    sched = preprocess(np.asarray(inputs["edge_index"]), cfg)
    nc = build_program(cfg, sched)
    in_maps = make_inputs(inputs["x"], inputs, cfg, sched)

    res = run_bass_kernel_spmd(
        nc, in_maps, core_ids=list(range(C)), trace=trace,
        stitch_traces=trace,
    )
    LAST_EXEC_NS = res.exec_time_ns
    LAST_RESULTS = res

    F_LAST = cfg["DIMS"][3]
    out = np.empty((N, F_LAST), np.float32)
    core_of, local_of = sched["core_of"], sched["local_of"]
    for c in range(C):
        nodes = np.where(core_of == c)[0]
        out[nodes] = res.results[c]["out_local"][local_of[nodes]]
    return out


def kernel(**inputs):
    return run(inputs, trace=bool(int(os.environ.get("GAT_TRACE", "0"))))

